# revision 1
# baseline (speedup 1.0000x reference)
"""Trainium2 Bass kernel for nn_DualSignalLinkPredictorC (2-layer GATv2 + MLP
link predictor), distributed over 8 NeuronCores.

Distribution (dst-sharded edge-parallel):
  - Nodes sharded 12500/core; edges (incl. self-loops) grouped by dst into
    128-node row-tiles, packed into 128-edge subtiles, bucketed by source
    table chunk (4 chunks) so int16 dma_gather indices stay in range.
  - Segment softmax + scatter-add are one-hot matmuls accumulating in PSUM.
    No segment-max pass: softmax is shift-invariant and scores are O(0.1),
    so exp() is stable without it; denominators are applied as a per-row
    scale after aggregation.
  - Gather tables (xl1 / xl2 / z) are bf16, built per-shard and AllGathered
    in 4 chunks; per-edge source rows fetched with one indirect DMA per
    128-edge subtile (int32 row indices, chunk-local).
  - Decode pairs are grouped by (src-chunk, dst-chunk) host-side so both
    sides gather with aligned slots; host un-permutes the result.
"""

import numpy as np
import ml_dtypes

BF16 = ml_dtypes.bfloat16


class Cfg:
    def __init__(self, N=100000, E=1600000, NPAIRS=262144, NC=8, NCH=4,
                 RAW=512, IN=256, HID=256, EMB=128, GATHER_GROUP=2):
        self.N, self.E, self.NPAIRS, self.NC, self.NCH = N, E, NPAIRS, NC, NCH
        self.RAW, self.IN, self.HID, self.EMB = RAW, IN, HID, EMB
        self.G = GATHER_GROUP
        assert N % NC == 0
        self.SH = N // NC
        assert self.SH % NCH == 0
        self.CH = self.SH // NCH          # AllGather chunk rows (per core)
        self.CHN = self.CH * NC           # table chunk rows (physical)
        assert self.CHN <= 32768, "dma_gather int16 index range"
        self.RT = (self.SH + 127) // 128
        self.PPC = NPAIRS // NC
        assert self.PPC % 128 == 0


CFG = Cfg()


def phys_row(n, cfg):
    """Physical row in the chunk-ordered AllGathered tables of global node n."""
    c = n // cfg.SH
    r = n - c * cfg.SH
    k = r // cfg.CH
    q = r - k * cfg.CH
    return k * cfg.CHN + c * cfg.CH + q


class EdgePlan:
    """Host-side packing of edges into (row-tile, chunk-bucket, subtile, slot).

    Device contract:
      - per gather group g (G row-tiles) and chunk k: one dma_gather of
        n_gk = 128 * sum_t S[t][k] slots into xg blocks; xg block order
        within a group is k-major, then tile, then subtile.
      - LIDP/LIDF columns are per row-tile in (k, subtile) order.
      - IDX16 is the int16 wrapped index stream, one region per (g, k).
    """

    def __init__(self, cfg, src_phys, dst):
        NC, SH, RT, NCH, G = cfg.NC, cfg.SH, cfg.RT, cfg.NCH, cfg.G
        self.cfg = cfg
        core_of = dst // SH
        r_in_core = dst - core_of * SH
        tile_of = r_in_core >> 7
        lid = (r_in_core & 127).astype(np.float32)
        chunk = src_phys // cfg.CHN
        loc = (src_phys - chunk * cfg.CHN).astype(np.int64)

        key = ((core_of * RT + tile_of) * NCH + chunk)
        order = np.argsort(key, kind="stable")
        counts = np.bincount(key, minlength=NC * RT * NCH).reshape(NC, RT, NCH)
        starts = np.concatenate([[0], np.cumsum(counts.ravel())])[:-1].reshape(NC, RT, NCH)

        S_tk = np.ceil(counts.max(axis=0) / 128).astype(np.int64)   # [RT, NCH]
        deg = np.bincount(dst, minlength=cfg.N)
        assert deg.max() <= 128, "in-degree > 128 unsupported"
        self.S_tk = S_tk
        self.S_t = S_tk.sum(axis=1)
        self.S_tot = int(self.S_t.sum())

        S_off = np.concatenate([[0], np.cumsum(self.S_t)]).astype(int)
        self.S_off = S_off
        IDX32 = np.zeros((NC, 128, self.S_tot), dtype=np.int32)
        LIDP = np.full((NC, 128, self.S_tot), 255.0, dtype=np.float32)
        for c in range(NC):
            for t in range(RT):
                for k in range(NCH):
                    for i in range(int(S_tk[t, k])):
                        n_e = counts[c, t, k]
                        lo = i * 128
                        m = int(min(128, max(0, n_e - lo)))
                        sl = order[starts[c, t, k] + lo:starts[c, t, k] + lo + m]
                        vals = np.zeros(128, dtype=np.int64)
                        lids = np.full(128, 255.0, dtype=np.float32)
                        vals[:m] = loc[sl]
                        lids[:m] = lid[sl]
                        jcol = S_off[t] + int(np.sum(S_tk[t, :k])) + i
                        IDX32[c, :, jcol] = vals
                        LIDP[c, :, jcol] = lids
        self.IDX32 = [np.ascontiguousarray(IDX32[c]) for c in range(NC)]
        self.LIDP = [np.ascontiguousarray(LIDP[c].astype(BF16)) for c in range(NC)]


class DecodePlan:
    """Group pairs by (ps_chunk, pd_chunk) per core; pad groups to x128."""

    def __init__(self, cfg, psp, pdp):
        NC, NCH, PPC = cfg.NC, cfg.NCH, cfg.PPC
        self.cfg = cfg
        pa = psp.reshape(NC, PPC)
        pb = pdp.reshape(NC, PPC)
        grp = (pa // cfg.CHN) * NCH + (pb // cfg.CHN)
        cnt = np.zeros((NC, NCH * NCH), dtype=np.int64)
        for c in range(NC):
            cnt[c] = np.bincount(grp[c], minlength=NCH * NCH)
        self.DZ = np.maximum((np.ceil(cnt.max(axis=0) / 128) * 128).astype(np.int64), 128)
        self.tot_slots = int(self.DZ.sum())
        self.g_off = np.concatenate([[0], np.cumsum(self.DZ)]).astype(int)

        PS32 = np.zeros((NC, 128, self.tot_slots // 128), dtype=np.int32)
        PD32 = np.zeros((NC, 128, self.tot_slots // 128), dtype=np.int32)
        self.perm = np.full((NC, self.tot_slots), -1, dtype=np.int64)
        for c in range(NC):
            for gidx in range(NCH * NCH):
                ids = np.nonzero(grp[c] == gidx)[0]
                o = self.g_off[gidx]
                s_ = o + np.arange(len(ids))
                PS32[c, s_ % 128, s_ // 128] = pa[c, ids] % cfg.CHN
                PD32[c, s_ % 128, s_ // 128] = pb[c, ids] % cfg.CHN
                self.perm[c, s_] = ids
        self.PS32 = [np.ascontiguousarray(PS32[c]) for c in range(NC)]
        self.PD32 = [np.ascontiguousarray(PD32[c]) for c in range(NC)]

    def unscramble(self, res_slots):
        cfg = self.cfg
        out = np.zeros(cfg.NPAIRS, dtype=np.float32)
        for c in range(cfg.NC):
            m = self.perm[c] >= 0
            out[c * cfg.PPC + self.perm[c][m]] = res_slots[c][m]
        return out


def host_prep(x, edge_index, edge_pairs, cfg):
    x = np.nan_to_num(np.asarray(x, dtype=np.float32), nan=0.0, posinf=0.0,
                      neginf=0.0)
    ei = np.asarray(edge_index, dtype=np.int64)
    ep = np.asarray(edge_pairs, dtype=np.int64)
    N = cfg.N

    src = np.concatenate([ei[0], np.arange(N, dtype=np.int64)])
    dst = np.concatenate([ei[1], np.arange(N, dtype=np.int64)])
    eplan = EdgePlan(cfg, phys_row(src, cfg), dst)
    dplan = DecodePlan(cfg, phys_row(ep[:, 0], cfg), phys_row(ep[:, 1], cfg))

    xT = [np.ascontiguousarray(x[c * cfg.SH:(c + 1) * cfg.SH].T.astype(BF16))
          for c in range(cfg.NC)]
    return eplan, dplan, xT


def prep_weights(inp, cfg):
    f = lambda a: np.asarray(a, np.float32)
    W = {}
    W["WpT"] = np.ascontiguousarray(f(inp["Wp"]).T.astype(BF16))
    for k in ("Wl1", "Wr1", "Wm1", "Wm2", "Wl2", "Wr2"):
        W[k + "T"] = np.ascontiguousarray(f(inp[k]).T.astype(BF16))
    W["ATT1R"] = np.ascontiguousarray(np.broadcast_to(
        f(inp["att1"]).reshape(1, -1), (128, cfg.HID))).astype(BF16)
    W["ATT2R"] = np.ascontiguousarray(np.broadcast_to(
        f(inp["att2"]).reshape(1, -1), (128, cfg.EMB))).astype(BF16)
    W["IDENT"] = np.ascontiguousarray(np.eye(128, dtype=np.float32).astype(BF16))
    W["IOTA_ROWS"] = np.ascontiguousarray(np.broadcast_to(
        np.arange(128, dtype=np.float32), (128, 128))).astype(BF16)
    W["IOTA_COL"] = np.ascontiguousarray(
        np.arange(128, dtype=np.float32)[:, None].astype(BF16))
    alpha = 1.0 / (1.0 + np.exp(-float(f(inp["logit_alpha"]).ravel()[0])))
    temp = float(f(inp["temperature"]))
    W["A12R"] = np.ascontiguousarray(np.broadcast_to(
        np.array([alpha * temp, (1.0 - alpha) * temp], np.float32), (128, 2))).copy()
    return W


# ---------------------------------------------------------------------------
# device program
# ---------------------------------------------------------------------------

def build_program(eplan, dplan, cfg, use_lrelu=False):
    import contextlib
    import concourse.bass as bass
    import concourse.tile as tile
    from concourse import bacc, mybir

    dt = mybir.dt
    AF = mybir.ActivationFunctionType
    OP = mybir.AluOpType
    AX = mybir.AxisListType

    NC, SH, RT, NCH, CH, CHN, G = (cfg.NC, cfg.SH, cfg.RT, cfg.NCH, cfg.CH,
                                   cfg.CHN, cfg.G)
    RAW, IN, HID, EMB = cfg.RAW, cfg.IN, cfg.HID, cfg.EMB
    KQ = RAW // 128
    S_tk, S_t, S_off = eplan.S_tk, eplan.S_t, eplan.S_off
    SMAX = int(S_t.max())
    EPS_LN = 1e-5
    EPS_DEN = 1e-16

    nc = bacc.Bacc("TRN2", target_bir_lowering=False, debug=False,
                   num_devices=NC)

    din = lambda name, shape, d: nc.dram_tensor(name, shape, d, kind="ExternalInput").ap()
    xT = din("xT", [RAW, SH], dt.bfloat16)
    IDX32 = din("IDX32", [128, eplan.S_tot], dt.int32)
    LIDP = din("LIDP", [128, eplan.S_tot], dt.bfloat16)
    PS32 = din("PS32", [128, dplan.tot_slots // 128], dt.int32)
    PD32 = din("PD32", [128, dplan.tot_slots // 128], dt.int32)
    WpT = din("WpT", [RAW, IN], dt.bfloat16)
    Wl1T = din("Wl1T", [IN, HID], dt.bfloat16)
    Wr1T = din("Wr1T", [IN, HID], dt.bfloat16)
    Wm1T = din("Wm1T", [IN, HID], dt.bfloat16)
    Wm2T = din("Wm2T", [HID, EMB], dt.bfloat16)
    Wl2T = din("Wl2T", [HID, EMB], dt.bfloat16)
    Wr2T = din("Wr2T", [HID, EMB], dt.bfloat16)
    ATT1R = din("ATT1R", [128, HID], dt.bfloat16)
    ATT2R = din("ATT2R", [128, EMB], dt.bfloat16)
    IDENT = din("IDENT", [128, 128], dt.bfloat16)
    IOTA_ROWS = din("IOTA_ROWS", [128, 128], dt.bfloat16)
    IOTA_COL = din("IOTA_COL", [128, 1], dt.bfloat16)
    A12R = din("A12R", [128, 2], dt.float32)

    res_out = nc.dram_tensor("res", [dplan.tot_slots], dt.float32,
                             kind="ExternalOutput").ap()

    rg = [list(range(NC))]
    # Per-phase parity semaphores for gather completion: row-tile t's gathers
    # all bump sem[t%2]; the fence waits for the cumulative count. Tiles t and
    # t+2 can't have gathers in flight together (xg buffer reuse WARs through
    # the fence), so each sem is quiesced when its next user starts.
    gsems = {ph: [nc.alloc_semaphore(f"gsem_{ph}{i}") for i in range(2)]
             for ph in ("a", "b", "d")}
    gcnt = {ph: [0, 0] for ph in ("a", "b", "d")}

    def rows(t):
        return min(128, SH - 128 * t)

    with tile.TileContext(nc) as tc:
        ctx = contextlib.ExitStack()
        with ctx:
            cpool = ctx.enter_context(tc.tile_pool(name="consts", bufs=1))
            dpool = ctx.enter_context(tc.tile_pool(name="dram", bufs=1, space="DRAM"))
            sstat = ctx.enter_context(tc.tile_pool(name="sstat", bufs=2))
            dsb = ctx.enter_context(tc.tile_pool(name="dsb", bufs=2))
            dps = ctx.enter_context(tc.tile_pool(name="dps", bufs=2, space="PSUM"))

            def cload(ap, shape, d=dt.bfloat16, name=None):
                t_ = cpool.tile(shape, d, name=name)
                nc.sync.dma_start(t_[:], ap)
                return t_

            wpT_s = cload(WpT.rearrange("(q p) o -> p q o", p=128), [128, KQ, IN], name="wpT_s")
            wl1_s = cload(Wl1T.rearrange("(q p) o -> p q o", p=128), [128, IN // 128, HID], name="wl1_s")
            wr1_s = cload(Wr1T.rearrange("(q p) o -> p q o", p=128), [128, IN // 128, HID], name="wr1_s")
            wm1_s = cload(Wm1T.rearrange("(q p) o -> p q o", p=128), [128, IN // 128, HID], name="wm1_s")
            wm2_s = cload(Wm2T.rearrange("(q p) o -> p q o", p=128), [128, HID // 128, EMB], name="wm2_s")
            wl2_s = cload(Wl2T.rearrange("(q p) o -> p q o", p=128), [128, HID // 128, EMB], name="wl2_s")
            wr2_s = cload(Wr2T.rearrange("(q p) o -> p q o", p=128), [128, HID // 128, EMB], name="wr2_s")
            att1_s = cload(ATT1R, [128, HID], name="att1_s")
            att2_s = cload(ATT2R, [128, EMB], name="att2_s")
            ident_s = cload(IDENT, [128, 128], name="ident_s")
            iotar_s = cload(IOTA_ROWS, [128, 128], name="iotar_s")
            iotac_s = cload(IOTA_COL, [128, 1], name="iotac_s")
            a12_s = cload(A12R, [128, 2], dt.float32, name="a12_s")

            xl1_own = dpool.tile([SH, HID], dt.bfloat16, name="xl1_own")
            xr1_own = dpool.tile([SH, HID], dt.bfloat16, name="xr1_own")
            xl2_own = dpool.tile([SH, EMB], dt.bfloat16, name="xl2_own")
            xr2_own = dpool.tile([SH, EMB], dt.bfloat16, name="xr2_own")
            z_own = dpool.tile([SH, 2 * EMB], dt.bfloat16, name="z_own")
            xl1_tbl = [dpool.tile([CHN, HID], dt.bfloat16, name=f"xl1_tbl{k}",
                                  addr_space="Shared") for k in range(NCH)]
            xl2_tbl = [dpool.tile([CHN, EMB], dt.bfloat16, name=f"xl2_tbl{k}",
                                  addr_space="Shared") for k in range(NCH)]
            z_tbl = [dpool.tile([CHN, 2 * EMB], dt.bfloat16, name=f"z_tbl{k}",
                                addr_space="Shared") for k in range(NCH)]

            # ---------------- helpers ----------------
            def layernorm_relu(src_t, n, D, out_bf):
                sm = sstat.tile([128, 1], dt.float32, name="sm", tag="sm")
                nc.vector.tensor_reduce(sm[:n], src_t[:n, :D], axis=AX.X, op=OP.add)
                scr = sstat.tile([128, 256], dt.float32, name="scr", tag="scr")
                sq = sstat.tile([128, 1], dt.float32, name="sq", tag="sq")
                nc.scalar.activation(scr[:n, :D], src_t[:n, :D], AF.Square,
                                     accum_out=sq[:n])
                mu = sstat.tile([128, 1], dt.float32, name="mu", tag="mu")
                nc.vector.tensor_scalar(out=mu[:n], in0=sm[:n], scalar1=1.0 / D,
                                        scalar2=None, op0=OP.mult)
                msq = sstat.tile([128, 1], dt.float32, name="msq", tag="msq")
                nc.vector.tensor_tensor(out=msq[:n], in0=mu[:n], in1=mu[:n], op=OP.mult)
                var = sstat.tile([128, 1], dt.float32, name="var", tag="var")
                nc.vector.scalar_tensor_tensor(out=var[:n], in0=sq[:n],
                                               scalar=1.0 / D, in1=msq[:n],
                                               op0=OP.mult, op1=OP.subtract)
                veps = sstat.tile([128, 1], dt.float32, name="veps", tag="veps")
                nc.vector.tensor_scalar(out=veps[:n], in0=var[:n], scalar1=EPS_LN,
                                        scalar2=None, op0=OP.add)
                rinv = sstat.tile([128, 1], dt.float32, name="rinv", tag="rinv")
                nc.vector.reciprocal(rinv[:n], veps[:n])
                rstd = sstat.tile([128, 1], dt.float32, name="rstd", tag="rstd")
                nc.scalar.activation(rstd[:n], rinv[:n], AF.Sqrt)
                nb = sstat.tile([128, 1], dt.float32, name="nb", tag="nb")
                nc.vector.scalar_tensor_tensor(out=nb[:n], in0=mu[:n], scalar=-1.0,
                                               in1=rstd[:n], op0=OP.mult, op1=OP.mult)
                nc.scalar.activation(out_bf[:n, :D], src_t[:n, :D], AF.Relu,
                                     bias=nb[:n], scale=rstd[:n])

            def transpose_to(src_bf, n, D, name):
                out = dsb.tile([128, D // 128, 128], dt.bfloat16, name=name,
                               tag=name, padded_shape=[128, 2, 128])
                for b in range(D // 128):
                    tp = dps.tile([128, 128], dt.bfloat16, name=name + "_ps",
                                  tag="tp", space="PSUM", bufs=1)
                    nc.tensor.transpose(tp[:, :n], src_bf[:n, 128 * b:128 * (b + 1)],
                                        ident_s[:n, :n])
                    nc.scalar.copy(out[:, b, :n], tp[:, :n])
                return out

            def proj(inT, n, wT, Dout, name, kchunks):
                ps_t = dps.tile([128, 256], dt.float32, name=name + "_ps",
                                tag="proj", space="PSUM", bufs=1)
                for q in range(kchunks):
                    nc.tensor.matmul(out=ps_t[:n, :Dout], lhsT=inT[:, q, :n],
                                     rhs=wT[:, q, :], start=(q == 0),
                                     stop=(q == kchunks - 1), skip_group_check=True)
                return ps_t

            # ================= dense phase =================
            for t in range(RT):
                n = rows(t)
                xt = dsb.tile([128, KQ, 128], dt.bfloat16, name="xt")
                nc.sync.dma_start(xt[:, :, :n],
                                  xT.rearrange("(q p) m -> p q m", p=128)[:, :, 128 * t:128 * t + n])
                xp_ps = proj(xt, n, wpT_s, IN, "xp", KQ)
                xp = dsb.tile([128, IN], dt.bfloat16, name="xp")
                layernorm_relu(xp_ps, n, IN, xp)
                xpT = transpose_to(xp, n, IN, "xpT")

                xl1_ps = proj(xpT, n, wl1_s, HID, "xl1", IN // 128)
                xl1_bf = dsb.tile([128, HID], dt.bfloat16, name="xl1_bf")
                nc.scalar.copy(xl1_bf[:n, :], xl1_ps[:n, :HID])
                nc.sync.dma_start(xl1_own[128 * t:128 * t + n, :], xl1_bf[:n, :])

                xr1_ps = proj(xpT, n, wr1_s, HID, "xr1", IN // 128)
                xr1_bf = dsb.tile([128, HID], dt.bfloat16, name="xr1_bf")
                nc.vector.tensor_copy(xr1_bf[:n, :], xr1_ps[:n, :HID])
                nc.sync.dma_start(xr1_own[128 * t:128 * t + n, :], xr1_bf[:n, :])

                m1_ps = proj(xpT, n, wm1_s, HID, "m1", IN // 128)
                m1 = dsb.tile([128, HID], dt.bfloat16, name="m1")
                layernorm_relu(m1_ps, n, HID, m1)
                m1T = transpose_to(m1, n, HID, "m1T")
                zf_ps = proj(m1T, n, wm2_s, EMB, "zf", HID // 128)
                zf_bf = dsb.tile([128, EMB], dt.bfloat16, name="zf_bf")
                nc.vector.tensor_copy(zf_bf[:n, :], zf_ps[:n, :EMB])
                nc.sync.dma_start(z_own[128 * t:128 * t + n, EMB:], zf_bf[:n, :])

            for k in range(NCH):
                nc.gpsimd.collective_compute(
                    "AllGather", OP.bypass, replica_groups=rg,
                    ins=[xl1_own[CH * k:CH * (k + 1), :].opt()],
                    outs=[xl1_tbl[k][:].opt()])

            # ================= edge phases =================
            def edge_tile(pools, t, xr_own_t, tbl, D, HEADS, att_s, out_cb, suf):
                esb, egat, eps_, epo = (pools["esb"], pools["egat"],
                                        pools["eps"], pools["epo"])
                n = rows(t)
                S = int(S_t[t])
                c0 = int(S_off[t])
                psem = gsems[suf][t % 2]

                idx_t = esb.tile([128, S], dt.int32, name=f"idx{suf}",
                                 tag=f"idx{suf}", padded_shape=[128, SMAX])
                nc.sync.dma_start(idx_t[:], IDX32[:, c0:c0 + S])
                lidp_t = esb.tile([128, S], dt.bfloat16, name=f"lidp{suf}",
                                  tag=f"lidp{suf}", padded_shape=[128, SMAX])
                nc.sync.dma_start(lidp_t[:], LIDP[:, c0:c0 + S])
                xr_t = esb.tile([128, D], dt.bfloat16, name=f"xr{suf}", tag=f"xr{suf}")
                nc.sync.dma_start(xr_t[:n, :], xr_own_t[128 * t:128 * t + n, :])

                # one indirect gather per subtile (128 rows each)
                xg = egat.tile([128, S * D], dt.bfloat16, name=f"xg{suf}",
                               tag=f"xg{suf}", padded_shape=[128, SMAX * D])
                subtiles = []
                for k in range(NCH):
                    for i in range(int(S_tk[t, k])):
                        subtiles.append((k, i))
                assert len(subtiles) == S
                for j, (k, i) in enumerate(subtiles):
                    nc.gpsimd.indirect_dma_start(
                        out=xg[:, j * D:(j + 1) * D], out_offset=None,
                        in_=tbl[k][:],
                        in_offset=bass.IndirectOffsetOnAxis(
                            ap=idx_t[:, j:j + 1], axis=0),
                    ).then_inc(psem, 16)
                gcnt[suf][t % 2] += S
                wv = 16 * gcnt[suf][t % 2]
                nc.gpsimd.tensor_copy(
                    xg[:1, :].rearrange("p (s d) -> p s d", d=D)[:, :, :1],
                    xg[:1, :].rearrange("p (s d) -> p s d", d=D)[:, :, :1],
                )._wait_ge(psem, wv)
                nc.gpsimd.tensor_copy(
                    idx_t[:1, :1], idx_t[:1, :1])._wait_ge(psem, wv)

                # wide one-op MT build; M via PE transpose per subtile
                mt_all = esb.tile([128, S, 128], dt.bfloat16, name=f"mt{suf}",
                                  tag=f"mt{suf}", padded_shape=[128, SMAX, 128])
                in0 = bass.AP(lidp_t.tensor, lidp_t.offset,
                              [list(lidp_t.ap[0]), [lidp_t.ap[1][0], S], [0, 128]])
                in1 = bass.AP(iotar_s.tensor, iotar_s.offset,
                              [list(iotar_s.ap[0]), [0, S], [1, 128]])
                nc.vector.tensor_tensor(out=mt_all[:, :S, :], in0=in0, in1=in1,
                                        op=OP.is_equal)
                m_all = esb.tile([128, S, 128], dt.bfloat16, name=f"m{suf}",
                                 tag=f"m{suf}", padded_shape=[128, SMAX, 128])
                for j in range(S):
                    mps = eps_.tile([128, 128], dt.bfloat16, name=f"mps{suf}",
                                    tag=f"mps{suf}", space="PSUM")
                    nc.tensor.transpose(mps[:], mt_all[:, j, :], ident_s[:])
                    nc.scalar.copy(m_all[:, j, :], mps[:])

                e_all = esb.tile([128, S * D], dt.bfloat16, name=f"eall{suf}",
                                 tag=f"eall{suf}", padded_shape=[128, SMAX * D])
                po = epo.tile([128, D + 8], dt.float32, name=f"po{suf}",
                              tag=f"po{suf}", space="PSUM")

                for j in range(S):
                    e_ps = eps_.tile([128, 256], dt.float32, name=f"e{suf}",
                                     tag=f"e{suf}", space="PSUM")
                    nc.tensor.matmul(out=e_ps[:, :D], lhsT=m_all[:n, j, :],
                                     rhs=xr_t[:n, :], start=True, stop=False,
                                     skip_group_check=True)
                    nc.tensor.matmul(out=e_ps[:, :D], lhsT=ident_s[:],
                                     rhs=xg[:, j * D:(j + 1) * D], start=False,
                                     stop=True, skip_group_check=True)
                    if use_lrelu:
                        nc.scalar.activation(e_all[:, j * D:(j + 1) * D],
                                             e_ps[:, :D], AF.Lrelu, alpha=0.2)
                    else:
                        etmp = esb.tile([128, D], dt.bfloat16, name=f"et{suf}",
                                        tag=f"et{suf}", bufs=3)
                        nc.scalar.copy(etmp[:, :D], e_ps[:, :D])
                        nc.vector.scalar_tensor_tensor(
                            out=e_all[:, j * D:(j + 1) * D], in0=etmp[:, :D],
                            scalar=0.2, in1=etmp[:, :D], op0=OP.mult, op1=OP.max)

                sm_all = esb.tile([128, S * D], dt.bfloat16, name=f"sm{suf}",
                                  tag=f"sm{suf}", padded_shape=[128, SMAX * D],
                                  bufs=1)
                att_b = bass.AP(att_s.tensor, att_s.offset,
                                [list(att_s.ap[0]), [0, S], [1, D]])
                nc.vector.tensor_tensor(out=sm_all[:, :S * D],
                                        in0=e_all[:, :S * D],
                                        in1=att_b, op=OP.mult)
                sc_all = esb.tile([128, S * HEADS], dt.float32, name=f"sc{suf}",
                                  tag=f"sc{suf}", padded_shape=[128, SMAX * HEADS],
                                  bufs=1)
                nc.vector.tensor_reduce(
                    out=sc_all[:, :S * HEADS],
                    in_=sm_all[:, :S * D].rearrange("p (sh c) -> p sh c", c=D // HEADS),
                    axis=AX.X, op=OP.add)
                # exs packs [exl (D) | ex (H)] per subtile so aggregation +
                # denominator are ONE matmul accumulation chain (one PSUM
                # group, one bank).
                DH = D + HEADS
                exs = esb.tile([128, S * DH], dt.bfloat16, name=f"exs{suf}",
                               tag=f"exs{suf}", padded_shape=[128, SMAX * (D + 8)])
                ex_out = bass.AP(exs.tensor, exs.offset + D,
                                 [list(exs.ap[0]), [DH, S], [1, HEADS]])
                nc.scalar.activation(ex_out, sc_all[:, :S * HEADS].rearrange(
                    "p (s h) -> p s h", h=HEADS), AF.Exp)
                exl_out = bass.AP(exs.tensor, exs.offset,
                                  [list(exs.ap[0]), [DH, S], [1, D]])
                exb = bass.AP(exs.tensor, exs.offset + D,
                              [list(exs.ap[0]), [DH, S], [1, HEADS], [0, D // HEADS]])
                nc.vector.tensor_tensor(
                    out=exl_out,
                    in0=xg[:, :S * D].rearrange("p (s d) -> p s d", d=D),
                    in1=exb, op=OP.mult)

                for j in range(S):
                    nc.tensor.matmul(out=po[:n, :DH], lhsT=mt_all[:, j, :n],
                                     rhs=exs[:, j * DH:(j + 1) * DH],
                                     start=(j == 0), stop=(j == S - 1),
                                     skip_group_check=True)

                den = sstat.tile([128, 8], dt.float32, name=f"den{suf}",
                                 tag=f"den{suf}")
                nc.vector.tensor_scalar(out=den[:n, :HEADS], in0=po[:n, D:D + HEADS],
                                        scalar1=EPS_DEN, scalar2=None, op0=OP.add)
                rec = sstat.tile([128, 8], dt.float32, name=f"rec{suf}",
                                 tag=f"rec{suf}")
                nc.vector.reciprocal(rec[:n, :HEADS], den[:n, :HEADS])
                out_cb(po, rec, n, t)

            # ---- layer 1 ----
            def l1_out(pools, po, rec, n, t):
                esb = pools["esb"]
                outf = esb.tile([128, HID], dt.float32, name="outf", tag="outf")
                nc.vector.tensor_tensor(out=outf[:n, :], in0=po[:n, :HID],
                                        in1=rec[:n, :4].to_broadcast([n, 4, 64]),
                                        op=OP.mult)
                h_bf = esb.tile([128, HID], dt.bfloat16, name="h_bf", tag="h_bf")
                layernorm_relu(outf, n, HID, h_bf)
                hT = transpose_to(h_bf, n, HID, "hT")
                xl2_ps = proj(hT, n, wl2_s, EMB, "xl2", HID // 128)
                xl2_bf = esb.tile([128, EMB], dt.bfloat16, name="xl2_bf", tag="xl2_bf")
                nc.scalar.copy(xl2_bf[:n, :], xl2_ps[:n, :EMB])
                nc.sync.dma_start(xl2_own[128 * t:128 * t + n, :], xl2_bf[:n, :])
                xr2_ps = proj(hT, n, wr2_s, EMB, "xr2", HID // 128)
                xr2_bf = esb.tile([128, EMB], dt.bfloat16, name="xr2_bf", tag="xr2_bf")
                nc.vector.tensor_copy(xr2_bf[:n, :], xr2_ps[:n, :EMB])
                nc.sync.dma_start(xr2_own[128 * t:128 * t + n, :], xr2_bf[:n, :])

            with tc.tile_pool(name="esb_a", bufs=2) as esb_a, \
                 tc.tile_pool(name="egat_a", bufs=2) as egat_a, \
                 tc.tile_pool(name="eps_a", bufs=2, space="PSUM") as eps_a, \
                 tc.tile_pool(name="epo_a", bufs=2, space="PSUM") as epo_a:
                pools = {"esb": esb_a, "egat": egat_a, "eps": eps_a, "epo": epo_a}
                for t in range(RT):
                    edge_tile(pools, t, xr1_own, xl1_tbl, HID, 4, att1_s,
                              lambda po, rec, n, t_: l1_out(pools, po, rec, n, t_),
                              "a")

            for k in range(NCH):
                nc.gpsimd.collective_compute(
                    "AllGather", OP.bypass, replica_groups=rg,
                    ins=[xl2_own[CH * k:CH * (k + 1), :].opt()],
                    outs=[xl2_tbl[k][:].opt()])

            # ---- layer 2 ----
            def l2_out(pools, po, rec, n, t):
                esb = pools["esb"]
                zg = esb.tile([128, EMB], dt.bfloat16, name="zg", tag="zg")
                nc.vector.tensor_tensor(out=zg[:n, :], in0=po[:n, :EMB],
                                        in1=rec[:n, :1].to_broadcast([n, EMB]),
                                        op=OP.mult)
                nc.sync.dma_start(z_own[128 * t:128 * t + n, :EMB], zg[:n, :])

            with tc.tile_pool(name="esb_b", bufs=2) as esb_b, \
                 tc.tile_pool(name="egat_b", bufs=2) as egat_b, \
                 tc.tile_pool(name="eps_b", bufs=2, space="PSUM") as eps_b, \
                 tc.tile_pool(name="epo_b", bufs=2, space="PSUM") as epo_b:
                pools = {"esb": esb_b, "egat": egat_b, "eps": eps_b, "epo": epo_b}
                for t in range(RT):
                    edge_tile(pools, t, xr2_own, xl2_tbl, EMB, 1, att2_s,
                              lambda po, rec, n, t_: l2_out(pools, po, rec, n, t_),
                              "b")

            for k in range(NCH):
                nc.gpsimd.collective_compute(
                    "AllGather", OP.bypass, replica_groups=rg,
                    ins=[z_own[CH * k:CH * (k + 1), :].opt()],
                    outs=[z_tbl[k][:].opt()])

            # ================= decode =================
            D2 = 2 * EMB
            DZM = int(dplan.DZ.max())
            res_sb = cpool.tile([128, dplan.tot_slots // 128], dt.float32,
                                name="res_sb")
            with tc.tile_pool(name="dec", bufs=2) as dec, \
                 tc.tile_pool(name="decg", bufs=2) as decg:
                for gidx in range(NCH * NCH):
                    dz = int(dplan.DZ[gidx])
                    ka, kb = gidx // NCH, gidx % NCH
                    oslot = int(dplan.g_off[gidx])
                    ocol = oslot // 128
                    ntile = dz // 128
                    psem = gsems["d"][gidx % 2]
                    pi_t = dec.tile([128, ntile], dt.int32, name="pi", tag="pi",
                                    padded_shape=[128, DZM // 128])
                    nc.sync.dma_start(pi_t[:], PS32[:, ocol:ocol + ntile])
                    pj_t = dec.tile([128, ntile], dt.int32, name="pj", tag="pj",
                                    padded_shape=[128, DZM // 128])
                    nc.sync.dma_start(pj_t[:], PD32[:, ocol:ocol + ntile])
                    za = decg.tile([128, ntile * D2], dt.bfloat16, name="za",
                                   tag="za", padded_shape=[128, DZM // 128 * D2])
                    zb = decg.tile([128, ntile * D2], dt.bfloat16, name="zb",
                                   tag="zb", padded_shape=[128, DZM // 128 * D2])
                    for j in range(ntile):
                        nc.gpsimd.indirect_dma_start(
                            out=za[:, j * D2:(j + 1) * D2], out_offset=None,
                            in_=z_tbl[ka][:],
                            in_offset=bass.IndirectOffsetOnAxis(
                                ap=pi_t[:, j:j + 1], axis=0)).then_inc(psem, 16)
                        nc.gpsimd.indirect_dma_start(
                            out=zb[:, j * D2:(j + 1) * D2], out_offset=None,
                            in_=z_tbl[kb][:],
                            in_offset=bass.IndirectOffsetOnAxis(
                                ap=pj_t[:, j:j + 1], axis=0)).then_inc(psem, 16)
                    gcnt["d"][gidx % 2] += 2 * ntile
                    wv = 16 * gcnt["d"][gidx % 2]
                    nc.gpsimd.tensor_copy(
                        za[:1, :].rearrange("p (s d) -> p s d", d=D2)[:, :ntile, :1],
                        za[:1, :].rearrange("p (s d) -> p s d", d=D2)[:, :ntile, :1],
                    )._wait_ge(psem, wv)
                    nc.gpsimd.tensor_copy(
                        zb[:1, :].rearrange("p (s d) -> p s d", d=D2)[:, :ntile, :1],
                        zb[:1, :].rearrange("p (s d) -> p s d", d=D2)[:, :ntile, :1],
                    )._wait_ge(psem, wv)
                    nc.gpsimd.tensor_copy(
                        pi_t[:1, :1], pi_t[:1, :1])._wait_ge(psem, wv)
                    nc.gpsimd.tensor_copy(
                        pj_t[:1, :1], pj_t[:1, :1])._wait_ge(psem, wv)

                    dots = dec.tile([128, ntile, 2], dt.float32, name="dots",
                                    tag="dots", padded_shape=[128, DZM // 128, 2])
                    sqa = dec.tile([128, ntile, 2], dt.float32, name="sqa",
                                   tag="sqa", padded_shape=[128, DZM // 128, 2])
                    sqb = dec.tile([128, ntile, 2], dt.float32, name="sqb",
                                   tag="sqb", padded_shape=[128, DZM // 128, 2])
                    scrd = dec.tile([128, EMB], dt.float32, name="scrd", tag="scrd",
                                    bufs=3)
                    for j in range(ntile):
                        for h in range(2):
                            sl = slice(j * D2 + h * EMB, j * D2 + (h + 1) * EMB)
                            nc.vector.scalar_tensor_tensor(
                                out=scrd[:, :], in0=za[:, sl], scalar=1.0,
                                in1=zb[:, sl], op0=OP.mult, op1=OP.mult,
                                accum_out=dots[:, j, h:h + 1])
                            nc.scalar.activation(scrd[:, :], za[:, sl], AF.Square,
                                                 accum_out=sqa[:, j, h:h + 1])
                            nc.scalar.activation(scrd[:, :], zb[:, sl], AF.Square,
                                                 accum_out=sqb[:, j, h:h + 1])
                    nn_ = dec.tile([128, ntile * 2], dt.float32, name="nn_", tag="nn_",
                                   padded_shape=[128, 2 * DZM // 128])
                    nc.vector.tensor_tensor(out=nn_[:, :ntile * 2],
                                            in0=sqa[:, :ntile, :], in1=sqb[:, :ntile, :],
                                            op=OP.mult)
                    rin = dec.tile([128, ntile * 2], dt.float32, name="rin", tag="rin",
                                   padded_shape=[128, 2 * DZM // 128])
                    nc.vector.reciprocal(rin[:, :ntile * 2], nn_[:, :ntile * 2])
                    rsq = dec.tile([128, ntile * 2], dt.float32, name="rsq", tag="rsq",
                                   padded_shape=[128, 2 * DZM // 128])
                    nc.scalar.activation(rsq[:, :ntile * 2], rin[:, :ntile * 2], AF.Sqrt)
                    cosv = dec.tile([128, ntile * 2], dt.float32, name="cosv", tag="cosv",
                                    padded_shape=[128, 2 * DZM // 128])
                    nc.vector.tensor_tensor(out=cosv[:, :ntile * 2],
                                            in0=dots[:, :ntile, :],
                                            in1=rsq[:, :ntile * 2], op=OP.mult)
                    wz = dec.tile([128, ntile * 2], dt.float32, name="wz", tag="wz",
                                  padded_shape=[128, 2 * DZM // 128])
                    a12b = bass.AP(a12_s.tensor, a12_s.offset,
                                   [list(a12_s.ap[0]), [0, ntile], [1, 2]])
                    nc.vector.tensor_tensor(out=wz[:, :ntile * 2],
                                            in0=cosv[:, :ntile * 2], in1=a12b,
                                            op=OP.mult)
                    nc.vector.tensor_reduce(
                        out=res_sb[:, ocol:ocol + ntile],
                        in_=wz[:, :ntile * 2].rearrange("p (a b) -> p a b", b=2),
                        axis=AX.X, op=OP.add)

            nc.sync.dma_start(res_out.rearrange("(a b) -> b a", b=128), res_sb[:])

    nc.compile()
    return nc


# ---------------------------------------------------------------------------
# entry point
# ---------------------------------------------------------------------------

def make_in_maps(eplan, dplan, xT, W, cfg):
    in_maps = []
    for c in range(cfg.NC):
        m = {"xT": xT[c], "IDX32": eplan.IDX32[c], "LIDP": eplan.LIDP[c],
             "PS32": dplan.PS32[c], "PD32": dplan.PD32[c]}
        for k in ("WpT", "Wl1T", "Wr1T", "Wm1T", "Wm2T", "Wl2T", "Wr2T",
                  "ATT1R", "ATT2R", "IDENT", "IOTA_ROWS", "IOTA_COL", "A12R"):
            m[k] = W[k]
        in_maps.append(m)
    return in_maps


def kernel(**inputs):
    cfg = CFG
    eplan, dplan, xT = host_prep(inputs["x"], inputs["edge_index"],
                                 inputs["edge_pairs"], cfg)
    W = prep_weights(inputs, cfg)
    nc = build_program(eplan, dplan, cfg)
    from concourse.bass_utils import run_bass_kernel_spmd
    in_maps = make_in_maps(eplan, dplan, xT, W, cfg)
    res = run_bass_kernel_spmd(nc, in_maps, core_ids=list(range(cfg.NC)))
    slots = np.stack([res.results[c]["res"] for c in range(cfg.NC)])
    return dplan.unscramble(slots).astype(np.float32)



# revision 2
# speedup vs baseline: 5.0564x; 5.0564x over previous
"""Trainium2 Bass kernel for nn_DualSignalLinkPredictorC (2-layer GATv2 + MLP
link predictor), distributed over 8 NeuronCores.

v2: degree-sorted CSR layout. The backend executes ~1 instruction per 45us
regardless of width, so the design minimizes instruction count:
  - dst nodes on partitions, neighbors padded along the free dim (K_t = max
    in-degree per 128-node tile). Per-core nodes are permuted by descending
    degree so K_t tracks the local mean (~18) instead of the global max.
  - per-edge attention, segment softmax, and aggregation are wide DVE ops
    over [128, K_t*D] tiles; no one-hot matmuls, no PE transposes in the
    edge phase. Gathers are one indirect DMA per neighbor column.
  - single AllGather per table (no chunking; int32 row indices).
  - x is uploaded in fp8 (e4m3) to halve the dominant input transfer.
"""

import numpy as np
import ml_dtypes

BF16 = ml_dtypes.bfloat16
FP8 = ml_dtypes.float8_e4m3fn


class Cfg:
    def __init__(self, N=100000, E=1600000, NPAIRS=262144, NC=8,
                 RAW=512, IN=256, HID=256, EMB=128):
        self.N, self.E, self.NPAIRS, self.NC = N, E, NPAIRS, NC
        self.RAW, self.IN, self.HID, self.EMB = RAW, IN, HID, EMB
        assert N % NC == 0
        self.SH = N // NC
        self.T = (self.SH + 127) // 128
        self.PPC = NPAIRS // NC
        assert self.PPC % 128 == 0


CFG = Cfg()


class Plan:
    """Degree-sorted CSR neighbor plan + decode indices, per core."""

    def __init__(self, cfg, x, edge_index, edge_pairs):
        NC, SH, T = cfg.NC, cfg.SH, cfg.T
        ei = np.asarray(edge_index, dtype=np.int64)
        ep = np.asarray(edge_pairs, dtype=np.int64)
        loops = np.arange(cfg.N, dtype=np.int64)
        src = np.concatenate([ei[0], loops])
        dst = np.concatenate([ei[1], loops])

        deg = np.bincount(dst, minlength=cfg.N)
        # per-core descending-degree permutation; global_row maps node id ->
        # row in the AllGathered tables (core-major, rank within core).
        self.perm = []           # per core: rank -> local node
        global_row = np.empty(cfg.N, dtype=np.int64)
        for c in range(NC):
            dloc = deg[c * SH:(c + 1) * SH]
            p = np.argsort(-dloc, kind="stable")
            self.perm.append(p)
            rank = np.empty(SH, dtype=np.int64)
            rank[p] = np.arange(SH)
            global_row[c * SH:(c + 1) * SH] = c * SH + rank
        self.global_row = global_row

        srcrow = global_row[src]
        core_of = dst // SH
        rank_of = global_row[dst] - core_of * SH

        self.IDXE, self.DEGT, self.K_t, self.OFF_t = [], [], [], []
        for c in range(NC):
            m = core_of == c
            r = rank_of[m]
            v = srcrow[m]
            order = np.argsort(r, kind="stable")
            r, v = r[order], v[order]
            dsorted = np.zeros(SH, dtype=np.int64)
            dsorted[:SH] = np.bincount(r, minlength=SH)
            starts = np.concatenate([[0], np.cumsum(dsorted)])
            K_t = np.zeros(T, dtype=np.int64)
            for t in range(T):
                K_t[t] = dsorted[128 * t:128 * (t + 1)].max()
            OFF = np.concatenate([[0], np.cumsum(K_t)]).astype(np.int64)
            CK = int(OFF[-1])
            idxe = np.zeros((128, CK), dtype=np.int32)
            slot_in_dst = np.arange(len(r)) - starts[r]
            tile_of = r >> 7
            p_of = r & 127
            col = OFF[tile_of] + slot_in_dst
            idxe[p_of, col] = v
            degt = np.zeros((128, T), dtype=np.float32)
            dpad = np.concatenate([dsorted, np.zeros(T * 128 - SH, np.int64)])
            degt[:, :] = dpad.reshape(T, 128).T
            self.IDXE.append(np.ascontiguousarray(idxe))
            self.DEGT.append(degt)
            self.K_t.append(K_t)
            self.OFF_t.append(OFF)
        self.CKmax = max(int(o[-1]) for o in self.OFF_t)
        self.KMAX = max(int(k.max()) for k in self.K_t)
        # pad every core's IDXE to CKmax columns so shapes match SPMD
        for c in range(NC):
            ck = self.IDXE[c].shape[1]
            if ck < self.CKmax:
                self.IDXE[c] = np.ascontiguousarray(np.pad(
                    self.IDXE[c], ((0, 0), (0, self.CKmax - ck))))

        # decode: pair i of core c -> (p=i%128, col=i//128)
        self.PS, self.PD = [], []
        pr = global_row[ep[:, 0]].reshape(NC, cfg.PPC)
        qr = global_row[ep[:, 1]].reshape(NC, cfg.PPC)
        for c in range(NC):
            ps = pr[c].reshape(cfg.PPC // 128, 128).T.astype(np.int32)
            pd = qr[c].reshape(cfg.PPC // 128, 128).T.astype(np.int32)
            self.PS.append(np.ascontiguousarray(ps))
            self.PD.append(np.ascontiguousarray(pd))

        x = np.nan_to_num(np.asarray(x, dtype=np.float32), nan=0.0,
                          posinf=0.0, neginf=0.0)
        self.xT = []
        for c in range(NC):
            xs = x[c * SH:(c + 1) * SH][self.perm[c]]
            self.xT.append(np.ascontiguousarray(xs.T.astype(FP8)))


def host_prep(x, edge_index, edge_pairs, cfg):
    return Plan(cfg, x, edge_index, edge_pairs)


def prep_weights(inp, cfg):
    f = lambda a: np.asarray(a, np.float32)
    W = {}
    W["WpT"] = np.ascontiguousarray(f(inp["Wp"]).T.astype(BF16))
    for k in ("Wl1", "Wr1", "Wm1", "Wm2", "Wl2", "Wr2"):
        W[k + "T"] = np.ascontiguousarray(f(inp[k]).T.astype(BF16))
    W["ATT1R"] = np.ascontiguousarray(np.broadcast_to(
        f(inp["att1"]).reshape(1, -1), (128, cfg.HID))).astype(BF16)
    W["ATT2R"] = np.ascontiguousarray(np.broadcast_to(
        f(inp["att2"]).reshape(1, -1), (128, cfg.EMB))).astype(BF16)
    W["IDENT"] = np.ascontiguousarray(np.eye(128, dtype=np.float32).astype(BF16))
    W["IOTA_ROWS"] = np.ascontiguousarray(np.broadcast_to(
        np.arange(128, dtype=np.float32), (128, 128))).copy()
    alpha = 1.0 / (1.0 + np.exp(-float(f(inp["logit_alpha"]).ravel()[0])))
    temp = float(f(inp["temperature"]))
    W["A12R"] = np.ascontiguousarray(np.broadcast_to(
        np.array([alpha * temp, (1.0 - alpha) * temp], np.float32),
        (128, 2))).copy()
    return W


# ---------------------------------------------------------------------------
# device program
# ---------------------------------------------------------------------------

def build_program(plan, cfg, stage=5):
    import contextlib
    import concourse.bass as bass
    import concourse.tile as tile
    from concourse import bacc, mybir

    dt = mybir.dt
    AF = mybir.ActivationFunctionType
    OP = mybir.AluOpType
    AX = mybir.AxisListType

    NC, SH, T = cfg.NC, cfg.SH, cfg.T
    RAW, IN, HID, EMB = cfg.RAW, cfg.IN, cfg.HID, cfg.EMB
    KQ = RAW // 128
    N8 = SH * NC
    CK = plan.CKmax
    KMAX = plan.KMAX
    K_t, OFF_t = plan.K_t[0], plan.OFF_t[0]   # identical structure per core?
    EPS_LN = 1e-5
    EPS_DEN = 1e-16

    # NOTE: K_t / OFF_t differ per core. SPMD emits ONE program, so loop
    # bounds must be core-independent: use the per-tile MAX over cores.
    K_t = np.stack([plan.K_t[c] for c in range(NC)]).max(axis=0)
    OFF = np.concatenate([[0], np.cumsum(K_t)]).astype(np.int64)
    assert OFF[-1] <= CK or True
    CKU = int(OFF[-1])

    nc = bacc.Bacc("TRN2", target_bir_lowering=False, debug=False,
                   num_devices=NC)

    din = lambda name, shape, d: nc.dram_tensor(name, shape, d,
                                                kind="ExternalInput").ap()
    xT = din("xT", [RAW, SH], dt.float8e4)
    IDXE = din("IDXE", [128, CKU], dt.int32)
    DEGT = din("DEGT", [128, T], dt.float32)
    PS32 = din("PS32", [128, cfg.PPC // 128], dt.int32)
    PD32 = din("PD32", [128, cfg.PPC // 128], dt.int32)
    WpT = din("WpT", [RAW, IN], dt.bfloat16)
    Wl1T = din("Wl1T", [IN, HID], dt.bfloat16)
    Wr1T = din("Wr1T", [IN, HID], dt.bfloat16)
    Wm1T = din("Wm1T", [IN, HID], dt.bfloat16)
    Wm2T = din("Wm2T", [HID, EMB], dt.bfloat16)
    Wl2T = din("Wl2T", [HID, EMB], dt.bfloat16)
    Wr2T = din("Wr2T", [HID, EMB], dt.bfloat16)
    ATT1R = din("ATT1R", [128, HID], dt.bfloat16)
    ATT2R = din("ATT2R", [128, EMB], dt.bfloat16)
    IDENT = din("IDENT", [128, 128], dt.bfloat16)
    IOTA_ROWS = din("IOTA_ROWS", [128, 128], dt.float32)
    A12R = din("A12R", [128, 2], dt.float32)

    res_out = nc.dram_tensor("res", [cfg.PPC], dt.float32,
                             kind="ExternalOutput").ap()

    rg = [list(range(NC))]
    sems = {ph: nc.alloc_semaphore(f"gsem_{ph}") for ph in ("a", "b", "d")}
    gcnt = {ph: 0 for ph in ("a", "b", "d")}

    def rows(t):
        return min(128, SH - 128 * t)

    with tile.TileContext(nc) as tc:
        ctx = contextlib.ExitStack()
        with ctx:
            cpool = ctx.enter_context(tc.tile_pool(name="consts", bufs=1))
            dpool = ctx.enter_context(tc.tile_pool(name="dram", bufs=1,
                                                   space="DRAM"))
            sstat = ctx.enter_context(tc.tile_pool(name="sstat", bufs=2))
            dps = ctx.enter_context(tc.tile_pool(name="dps", bufs=2,
                                                 space="PSUM"))

            def cload(ap, shape, d=dt.bfloat16, name=None):
                t_ = cpool.tile(shape, d, name=name)
                nc.sync.dma_start(t_[:], ap)
                return t_

            wpT_s = cload(WpT.rearrange("(q p) o -> p q o", p=128),
                          [128, KQ, IN], name="wpT_s")
            wl1_s = cload(Wl1T.rearrange("(q p) o -> p q o", p=128),
                          [128, IN // 128, HID], name="wl1_s")
            wr1_s = cload(Wr1T.rearrange("(q p) o -> p q o", p=128),
                          [128, IN // 128, HID], name="wr1_s")
            wm1_s = cload(Wm1T.rearrange("(q p) o -> p q o", p=128),
                          [128, IN // 128, HID], name="wm1_s")
            wm2_s = cload(Wm2T.rearrange("(q p) o -> p q o", p=128),
                          [128, HID // 128, EMB], name="wm2_s")
            wl2_s = cload(Wl2T.rearrange("(q p) o -> p q o", p=128),
                          [128, HID // 128, EMB], name="wl2_s")
            wr2_s = cload(Wr2T.rearrange("(q p) o -> p q o", p=128),
                          [128, HID // 128, EMB], name="wr2_s")
            att1_s = cload(ATT1R, [128, HID], name="att1_s")
            att2_s = cload(ATT2R, [128, EMB], name="att2_s")
            ident_s = cload(IDENT, [128, 128], name="ident_s")
            iota_s = cload(IOTA_ROWS, [128, 128], dt.float32, name="iota_s")
            a12_s = cload(A12R, [128, 2], dt.float32, name="a12_s")
            deg_s = cload(DEGT, [128, T], dt.float32, name="deg_s")
            idxe_s = cload(IDXE, [128, CKU], dt.int32, name="idxe_s")

            xr1_all = cpool.tile([128, T, IN], dt.bfloat16, name="xr1_all")
            xr2_all = cpool.tile([128, T, EMB], dt.bfloat16, name="xr2_all")

            xl1_own = dpool.tile([SH, HID], dt.bfloat16, name="xl1_own")
            xl2_own = dpool.tile([SH, EMB], dt.bfloat16, name="xl2_own")
            z_own = dpool.tile([SH, 2 * EMB], dt.bfloat16, name="z_own")
            xl1_tbl = dpool.tile([N8, HID], dt.bfloat16, name="xl1_tbl",
                                 addr_space="Shared")
            xl2_tbl = dpool.tile([N8, EMB], dt.bfloat16, name="xl2_tbl",
                                 addr_space="Shared")
            z_tbl = dpool.tile([N8, 2 * EMB], dt.bfloat16, name="z_tbl",
                               addr_space="Shared")

            # -------- helpers --------
            def ln_relu(src_t, n, D, out_bf):
                """out = relu(layer_norm(src)); scale-invariant in src."""
                sm = sstat.tile([128, 1], dt.float32, name="sm", tag="sm")
                nc.vector.tensor_reduce(sm[:n], src_t[:n, :D], axis=AX.X,
                                        op=OP.add)
                scr = sstat.tile([128, 256], dt.float32, name="scr", tag="scr")
                sq = sstat.tile([128, 1], dt.float32, name="sq", tag="sq")
                nc.scalar.activation(scr[:n, :D], src_t[:n, :D], AF.Square,
                                     accum_out=sq[:n])
                msq = sstat.tile([128, 1], dt.float32, name="msq", tag="msq")
                nc.vector.scalar_tensor_tensor(out=msq[:n], in0=sm[:n],
                                               scalar=1.0 / (D * D),
                                               in1=sm[:n], op0=OP.mult,
                                               op1=OP.mult)
                var = sstat.tile([128, 1], dt.float32, name="var", tag="var")
                nc.vector.scalar_tensor_tensor(out=var[:n], in0=sq[:n],
                                               scalar=1.0 / D, in1=msq[:n],
                                               op0=OP.mult, op1=OP.subtract)
                veps = sstat.tile([128, 1], dt.float32, name="veps", tag="veps")
                nc.vector.tensor_scalar(out=veps[:n], in0=var[:n],
                                        scalar1=EPS_LN, scalar2=None,
                                        op0=OP.add)
                rinv = sstat.tile([128, 1], dt.float32, name="rinv", tag="rinv")
                nc.vector.reciprocal(rinv[:n], veps[:n])
                rstd = sstat.tile([128, 1], dt.float32, name="rstd", tag="rstd")
                nc.scalar.activation(rstd[:n], rinv[:n], AF.Sqrt)
                nb = sstat.tile([128, 1], dt.float32, name="nb", tag="nb")
                nc.vector.scalar_tensor_tensor(out=nb[:n], in0=sm[:n],
                                               scalar=-1.0 / D, in1=rstd[:n],
                                               op0=OP.mult, op1=OP.mult)
                nc.scalar.activation(out_bf[:n, :D], src_t[:n, :D], AF.Relu,
                                     bias=nb[:n], scale=rstd[:n])

            def transpose_to(pool, src_bf, n, D, name):
                out = pool.tile([128, D // 128, 128], dt.bfloat16, name=name,
                                tag=name, padded_shape=[128, 2, 128])
                for b in range(D // 128):
                    tp = dps.tile([128, 128], dt.bfloat16, name=name + "_ps",
                                  tag="tp", space="PSUM", bufs=1)
                    nc.tensor.transpose(tp[:, :n],
                                        src_bf[:n, 128 * b:128 * (b + 1)],
                                        ident_s[:n, :n])
                    nc.scalar.copy(out[:, b, :n], tp[:, :n])
                return out

            def proj(inT, n, wT, Dout, name, kchunks):
                ps_t = dps.tile([128, 256], dt.float32, name=name + "_ps",
                                tag="proj", space="PSUM", bufs=1)
                for q in range(kchunks):
                    nc.tensor.matmul(out=ps_t[:n, :Dout], lhsT=inT[:, q, :n],
                                     rhs=wT[:, q, :], start=(q == 0),
                                     stop=(q == kchunks - 1),
                                     skip_group_check=True)
                return ps_t

            # ================= dense phase =================
            with tc.tile_pool(name="dsb", bufs=2) as dsb:
                for t in range(T):
                    n = rows(t)
                    xt = dsb.tile([128, KQ, 128], dt.float8e4, name="xt")
                    nc.sync.dma_start(
                        xt[:, :, :n],
                        xT.rearrange("(q p) m -> p q m", p=128)[:, :, 128 * t:128 * t + n])
                    xp_ps = proj(xt, n, wpT_s, IN, "xp", KQ)
                    xp = dsb.tile([128, IN], dt.bfloat16, name="xp")
                    ln_relu(xp_ps, n, IN, xp)
                    xpT = transpose_to(dsb, xp, n, IN, "xpT")

                    xl1_ps = proj(xpT, n, wl1_s, HID, "xl1", IN // 128)
                    xl1_bf = dsb.tile([128, HID], dt.bfloat16, name="xl1_bf")
                    nc.scalar.copy(xl1_bf[:n, :], xl1_ps[:n, :HID])
                    nc.sync.dma_start(xl1_own[128 * t:128 * t + n, :],
                                      xl1_bf[:n, :])

                    xr1_ps = proj(xpT, n, wr1_s, HID, "xr1", IN // 128)
                    nc.vector.tensor_copy(xr1_all[:n, t, :], xr1_ps[:n, :HID])

                    m1_ps = proj(xpT, n, wm1_s, HID, "m1", IN // 128)
                    m1 = dsb.tile([128, HID], dt.bfloat16, name="m1")
                    ln_relu(m1_ps, n, HID, m1)
                    m1T = transpose_to(dsb, m1, n, HID, "m1T")
                    zf_ps = proj(m1T, n, wm2_s, EMB, "zf", HID // 128)
                    zf_bf = dsb.tile([128, EMB], dt.bfloat16, name="zf_bf")
                    nc.vector.tensor_copy(zf_bf[:n, :], zf_ps[:n, :EMB])
                    nc.sync.dma_start(z_own[128 * t:128 * t + n, EMB:],
                                      zf_bf[:n, :])

            if stage >= 2:
                nc.gpsimd.collective_compute(
                    "AllGather", OP.bypass, replica_groups=rg,
                    ins=[xl1_own[:].opt()], outs=[xl1_tbl[:].opt()])

            # ================= edge phase (CSR wide ops) =================
            def edge_tile(pools, t, xr_all, tbl, D, H, att_s, out_cb, suf):
                esb = pools["esb"]
                n = rows(t)
                Kt = int(K_t[t])
                c0 = int(OFF[t])
                DH = D // H
                psem = sems[suf]

                xg = esb.tile([128, Kt * D], dt.bfloat16, name=f"xg{suf}",
                              tag=f"xg{suf}", padded_shape=[128, KMAX * D])
                for k in range(Kt):
                    nc.gpsimd.indirect_dma_start(
                        out=xg[:, k * D:(k + 1) * D], out_offset=None,
                        in_=tbl[:],
                        in_offset=bass.IndirectOffsetOnAxis(
                            ap=idxe_s[:, c0 + k:c0 + k + 1], axis=0),
                    ).then_inc(psem, 16)
                gcnt[suf] += Kt
                nc.vector.tensor_copy(xg[:1, :1], xg[:1, :1])._wait_ge(
                    psem, 16 * gcnt[suf])

                # pad mask: 1.0 where k >= deg (padded slot)
                mpad = esb.tile([128, Kt], dt.float32, name=f"mp{suf}",
                                tag=f"mp{suf}", padded_shape=[128, KMAX])
                nc.vector.tensor_tensor(
                    out=mpad[:, :Kt], in0=iota_s[:, :Kt],
                    in1=bass.AP(deg_s.tensor, deg_s.offset + t,
                                [list(deg_s.ap[0]), [0, Kt]]),
                    op=OP.is_ge)

                # e = lrelu(xg + xr[dst]) ; score = <e, att> per head
                e_t = esb.tile([128, Kt * D], dt.bfloat16, name=f"e{suf}",
                               tag=f"e{suf}", padded_shape=[128, KMAX * D])
                xr_b = bass.AP(xr_all.tensor, xr_all.offset + t * D,
                               [list(xr_all.ap[0]), [0, Kt], [1, D]])
                nc.vector.tensor_tensor(out=e_t[:, :Kt * D],
                                        in0=xg[:, :Kt * D], in1=xr_b,
                                        op=OP.add)
                e2_t = esb.tile([128, Kt * D], dt.bfloat16, name=f"e2{suf}",
                                tag=f"e2{suf}", padded_shape=[128, KMAX * D])
                nc.vector.scalar_tensor_tensor(
                    out=e2_t[:, :Kt * D], in0=e_t[:, :Kt * D], scalar=0.2,
                    in1=e_t[:, :Kt * D], op0=OP.mult, op1=OP.max)
                att_b = bass.AP(att_s.tensor, att_s.offset,
                                [list(att_s.ap[0]), [0, Kt], [1, D]])
                sm_t = esb.tile([128, Kt * D], dt.bfloat16, name=f"smt{suf}",
                                tag=f"e{suf}", padded_shape=[128, KMAX * D])
                nc.vector.tensor_tensor(out=sm_t[:, :Kt * D],
                                        in0=e2_t[:, :Kt * D], in1=att_b,
                                        op=OP.mult)
                sc = esb.tile([128, Kt * H], dt.float32, name=f"sc{suf}",
                              tag=f"sc{suf}", padded_shape=[128, KMAX * H])
                nc.vector.tensor_reduce(
                    out=sc[:, :Kt * H],
                    in_=bass.AP(sm_t.tensor, sm_t.offset,
                                [list(sm_t.ap[0]), [DH, Kt * H], [1, DH]]),
                    axis=AX.X, op=OP.add)
                # mask pad slots: score += -100 * mpad   (broadcast over heads)
                mpad_b = bass.AP(mpad.tensor, mpad.offset,
                                 [list(mpad.ap[0]), [1, Kt], [0, H]])
                nc.vector.scalar_tensor_tensor(
                    out=sc[:, :Kt * H], in0=mpad_b, scalar=-100.0,
                    in1=sc[:, :Kt * H], op0=OP.mult, op1=OP.add)
                ex = esb.tile([128, Kt * H], dt.float32, name=f"ex{suf}",
                              tag=f"ex{suf}", padded_shape=[128, KMAX * H])
                nc.scalar.activation(ex[:, :Kt * H], sc[:, :Kt * H], AF.Exp)
                den = sstat.tile([128, 8], dt.float32, name=f"den{suf}",
                                 tag=f"den{suf}")
                nc.vector.tensor_reduce(
                    out=den[:, :H],
                    in_=bass.AP(ex.tensor, ex.offset,
                                [list(ex.ap[0]), [1, H], [H, Kt]]),
                    axis=AX.X, op=OP.add)
                dene = sstat.tile([128, 8], dt.float32, name=f"dene{suf}",
                                  tag=f"dene{suf}")
                nc.vector.tensor_scalar(out=dene[:, :H], in0=den[:, :H],
                                        scalar1=EPS_DEN, scalar2=None,
                                        op0=OP.add)
                rec = sstat.tile([128, 8], dt.float32, name=f"rec{suf}",
                                 tag=f"rec{suf}")
                nc.vector.reciprocal(rec[:, :H], dene[:, :H])
                alp = esb.tile([128, Kt * H], dt.bfloat16, name=f"al{suf}",
                               tag=f"al{suf}", padded_shape=[128, KMAX * H])
                rec_b = bass.AP(rec.tensor, rec.offset,
                                [list(rec.ap[0]), [0, Kt], [1, H]])
                nc.vector.tensor_tensor(out=alp[:, :Kt * H],
                                        in0=ex[:, :Kt * H], in1=rec_b,
                                        op=OP.mult)
                # w = xg * alpha ; out = sum_k w
                w_t = esb.tile([128, Kt * D], dt.bfloat16, name=f"w{suf}",
                               tag=f"e2{suf}", padded_shape=[128, KMAX * D])
                alp_b = bass.AP(alp.tensor, alp.offset,
                                [list(alp.ap[0]), [H, Kt], [1, H], [0, DH]])
                nc.vector.tensor_tensor(out=w_t[:, :Kt * D],
                                        in0=xg[:, :Kt * D], in1=alp_b,
                                        op=OP.mult)
                outf = esb.tile([128, D], dt.float32, name=f"o{suf}",
                                tag=f"o{suf}")
                nc.vector.tensor_reduce(
                    out=outf[:, :D],
                    in_=bass.AP(w_t.tensor, w_t.offset,
                                [list(w_t.ap[0]), [1, D], [D, Kt]]),
                    axis=AX.X, op=OP.add)
                out_cb(pools, outf, n, t)

            def l1_out(pools, outf, n, t):
                esb = pools["esb"]
                h_bf = esb.tile([128, HID], dt.bfloat16, name="h_bf",
                                tag="h_bf")
                ln_relu(outf, n, HID, h_bf)
                hT = transpose_to(esb, h_bf, n, HID, "hT")
                xl2_ps = proj(hT, n, wl2_s, EMB, "xl2", HID // 128)
                xl2_bf = esb.tile([128, EMB], dt.bfloat16, name="xl2_bf",
                                  tag="xl2_bf")
                nc.scalar.copy(xl2_bf[:n, :], xl2_ps[:n, :EMB])
                nc.sync.dma_start(xl2_own[128 * t:128 * t + n, :],
                                  xl2_bf[:n, :])
                xr2_ps = proj(hT, n, wr2_s, EMB, "xr2", HID // 128)
                nc.vector.tensor_copy(xr2_all[:n, t, :], xr2_ps[:n, :EMB])

            def l2_out(pools, outf, n, t):
                esb = pools["esb"]
                zg = esb.tile([128, EMB], dt.bfloat16, name="zg", tag="zg")
                nc.vector.tensor_copy(zg[:n, :], outf[:n, :EMB])
                nc.sync.dma_start(z_own[128 * t:128 * t + n, :EMB], zg[:n, :])

            if stage >= 3:
                with tc.tile_pool(name="esb_a", bufs=1) as esb_a:
                    pools = {"esb": esb_a}
                    for t in range(T):
                        edge_tile(pools, t, xr1_all, xl1_tbl, HID, 4, att1_s,
                                  l1_out, "a")

            if stage >= 4:
                nc.gpsimd.collective_compute(
                    "AllGather", OP.bypass, replica_groups=rg,
                    ins=[xl2_own[:].opt()], outs=[xl2_tbl[:].opt()])

                with tc.tile_pool(name="esb_b", bufs=1) as esb_b:
                    pools = {"esb": esb_b}
                    for t in range(T):
                        edge_tile(pools, t, xr2_all, xl2_tbl, EMB, 1, att2_s,
                                  l2_out, "b")

                nc.gpsimd.collective_compute(
                    "AllGather", OP.bypass, replica_groups=rg,
                    ins=[z_own[:].opt()], outs=[z_tbl[:].opt()])

            # ================= decode =================
            D2 = 2 * EMB
            NCOL = cfg.PPC // 128          # 256
            CC = 32                        # columns per chunk
            res_sb = cpool.tile([128, NCOL], dt.float32, name="res_sb")
            if stage < 5:
                nc.vector.memset(res_sb[:], 0.0)
            with tc.tile_pool(name="dec", bufs=1) as dec:
                if stage >= 5:
                    pi_t = cpool.tile([128, NCOL], dt.int32, name="pi")
                    nc.sync.dma_start(pi_t[:], PS32)
                    pj_t = cpool.tile([128, NCOL], dt.int32, name="pj")
                    nc.sync.dma_start(pj_t[:], PD32)
                for ch in range(NCOL // CC if stage >= 5 else 0):
                    o = ch * CC
                    za = dec.tile([128, CC * D2], dt.bfloat16, name="za",
                                  tag="za")
                    zb = dec.tile([128, CC * D2], dt.bfloat16, name="zb",
                                  tag="zb")
                    for j in range(CC):
                        nc.gpsimd.indirect_dma_start(
                            out=za[:, j * D2:(j + 1) * D2], out_offset=None,
                            in_=z_tbl[:],
                            in_offset=bass.IndirectOffsetOnAxis(
                                ap=pi_t[:, o + j:o + j + 1], axis=0),
                        ).then_inc(sems["d"], 16)
                        nc.gpsimd.indirect_dma_start(
                            out=zb[:, j * D2:(j + 1) * D2], out_offset=None,
                            in_=z_tbl[:],
                            in_offset=bass.IndirectOffsetOnAxis(
                                ap=pj_t[:, o + j:o + j + 1], axis=0),
                        ).then_inc(sems["d"], 16)
                    gcnt["d"] += 2 * CC
                    nc.vector.tensor_copy(za[:1, :1], za[:1, :1])._wait_ge(
                        sems["d"], 16 * gcnt["d"])
                    nc.vector.tensor_copy(zb[:1, :1], zb[:1, :1])._wait_ge(
                        sems["d"], 16 * gcnt["d"])

                    prod = dec.tile([128, CC * D2], dt.float32, name="prod",
                                    tag="prod")
                    view = lambda t_: bass.AP(
                        t_.tensor, t_.offset,
                        [list(t_.ap[0]), [EMB, CC * 2], [1, EMB]])
                    dots = dec.tile([128, CC * 2], dt.float32, name="dots",
                                    tag="dots")
                    nc.vector.tensor_tensor(out=prod[:], in0=za[:], in1=zb[:],
                                            op=OP.mult)
                    nc.vector.tensor_reduce(out=dots[:], in_=view(prod),
                                            axis=AX.X, op=OP.add)
                    sqa = dec.tile([128, CC * 2], dt.float32, name="sqa",
                                   tag="sqa")
                    nc.vector.tensor_tensor(out=prod[:], in0=za[:], in1=za[:],
                                            op=OP.mult)
                    nc.vector.tensor_reduce(out=sqa[:], in_=view(prod),
                                            axis=AX.X, op=OP.add)
                    sqb = dec.tile([128, CC * 2], dt.float32, name="sqb",
                                   tag="sqb")
                    nc.vector.tensor_tensor(out=prod[:], in0=zb[:], in1=zb[:],
                                            op=OP.mult)
                    nc.vector.tensor_reduce(out=sqb[:], in_=view(prod),
                                            axis=AX.X, op=OP.add)
                    nn_ = dec.tile([128, CC * 2], dt.float32, name="nn_",
                                   tag="nn_")
                    nc.vector.tensor_tensor(out=nn_[:], in0=sqa[:],
                                            in1=sqb[:], op=OP.mult)
                    rin = dec.tile([128, CC * 2], dt.float32, name="rin",
                                   tag="rin")
                    nc.vector.reciprocal(rin[:], nn_[:])
                    rsq = dec.tile([128, CC * 2], dt.float32, name="rsq",
                                   tag="rsq")
                    nc.scalar.activation(rsq[:], rin[:], AF.Sqrt)
                    cosv = dec.tile([128, CC * 2], dt.float32, name="cosv",
                                    tag="cosv")
                    nc.vector.tensor_tensor(out=cosv[:], in0=dots[:],
                                            in1=rsq[:], op=OP.mult)
                    wz = dec.tile([128, CC * 2], dt.float32, name="wz",
                                  tag="wz")
                    a12b = bass.AP(a12_s.tensor, a12_s.offset,
                                   [list(a12_s.ap[0]), [0, CC], [1, 2]])
                    nc.vector.tensor_tensor(out=wz[:], in0=cosv[:], in1=a12b,
                                            op=OP.mult)
                    nc.vector.tensor_reduce(
                        out=res_sb[:, o:o + CC],
                        in_=bass.AP(wz.tensor, wz.offset,
                                    [list(wz.ap[0]), [2, CC], [1, 2]]),
                        axis=AX.X, op=OP.add)

            nc.sync.dma_start(res_out.rearrange("(a b) -> b a", b=128),
                              res_sb[:])

    nc.compile()
    return nc


# ---------------------------------------------------------------------------
# entry point
# ---------------------------------------------------------------------------

def make_in_maps(plan, W, cfg):
    in_maps = []
    CKU = None
    for c in range(cfg.NC):
        m = {"xT": plan.xT[c], "DEGT": plan.DEGT[c],
             "PS32": plan.PS[c], "PD32": plan.PD[c]}
        for k in ("WpT", "Wl1T", "Wr1T", "Wm1T", "Wm2T", "Wl2T", "Wr2T",
                  "ATT1R", "ATT2R", "IDENT", "IOTA_ROWS", "A12R"):
            m[k] = W[k]
        in_maps.append(m)
    return in_maps


def finish_in_maps(in_maps, plan, cfg, nc):
    """Re-pack IDXE per core to the unified per-tile offsets of the program."""
    K_t = np.stack([plan.K_t[c] for c in range(cfg.NC)]).max(axis=0)
    OFF = np.concatenate([[0], np.cumsum(K_t)]).astype(np.int64)
    CKU = int(OFF[-1])
    for c in range(cfg.NC):
        idxe = np.zeros((128, CKU), dtype=np.int32)
        for t in range(cfg.T):
            kc = int(plan.K_t[c][t])
            oc = int(plan.OFF_t[c][t])
            idxe[:, int(OFF[t]):int(OFF[t]) + kc] = \
                plan.IDXE[c][:, oc:oc + kc]
        in_maps[c]["IDXE"] = idxe
    return in_maps


def kernel(**inputs):
    cfg = CFG
    plan = host_prep(inputs["x"], inputs["edge_index"],
                     inputs["edge_pairs"], cfg)
    W = prep_weights(inputs, cfg)
    nc = build_program(plan, cfg)
    from concourse.bass_utils import run_bass_kernel_spmd
    in_maps = finish_in_maps(make_in_maps(plan, W, cfg), plan, cfg, nc)
    res = run_bass_kernel_spmd(nc, in_maps, core_ids=list(range(cfg.NC)))
    out = np.concatenate([np.asarray(res.results[c]["res"])
                          for c in range(cfg.NC)])
    return out.astype(np.float32)


# revision 6
# speedup vs baseline: 5.2162x; 1.0316x over previous
"""Trainium2 Bass kernel for nn_DualSignalLinkPredictorC (2-layer GATv2 + MLP
link predictor), distributed over 8 NeuronCores.

v2: degree-sorted CSR layout. The backend executes ~1 instruction per 45us
regardless of width, so the design minimizes instruction count:
  - dst nodes on partitions, neighbors padded along the free dim (K_t = max
    in-degree per 128-node tile). Per-core nodes are permuted by descending
    degree so K_t tracks the local mean (~18) instead of the global max.
  - per-edge attention, segment softmax, and aggregation are wide DVE ops
    over [128, K_t*D] tiles; no one-hot matmuls, no PE transposes in the
    edge phase. Gathers are one indirect DMA per neighbor column.
  - single AllGather per table (no chunking; int32 row indices).
  - x is uploaded in fp8 (e4m3) to halve the dominant input transfer.
"""

import numpy as np
import ml_dtypes

BF16 = ml_dtypes.bfloat16
FP8 = ml_dtypes.float8_e4m3fn


class Cfg:
    def __init__(self, N=100000, E=1600000, NPAIRS=262144, NC=8,
                 RAW=512, IN=256, HID=256, EMB=128):
        self.N, self.E, self.NPAIRS, self.NC = N, E, NPAIRS, NC
        self.RAW, self.IN, self.HID, self.EMB = RAW, IN, HID, EMB
        assert N % NC == 0
        self.SH = N // NC
        self.T = (self.SH + 127) // 128
        self.PPC = NPAIRS // NC
        assert self.PPC % 128 == 0


CFG = Cfg()


class Plan:
    """Degree-sorted CSR neighbor plan + decode indices, per core."""

    def __init__(self, cfg, x, edge_index, edge_pairs):
        NC, SH, T = cfg.NC, cfg.SH, cfg.T
        ei = np.asarray(edge_index, dtype=np.int64)
        ep = np.asarray(edge_pairs, dtype=np.int64)
        loops = np.arange(cfg.N, dtype=np.int64)
        src = np.concatenate([ei[0], loops])
        dst = np.concatenate([ei[1], loops])

        deg = np.bincount(dst, minlength=cfg.N)
        # per-core descending-degree permutation; global_row maps node id ->
        # row in the AllGathered tables (core-major, rank within core).
        self.perm = []           # per core: rank -> local node
        global_row = np.empty(cfg.N, dtype=np.int64)
        for c in range(NC):
            dloc = deg[c * SH:(c + 1) * SH]
            p = np.argsort(-dloc, kind="stable")
            self.perm.append(p)
            rank = np.empty(SH, dtype=np.int64)
            rank[p] = np.arange(SH)
            global_row[c * SH:(c + 1) * SH] = c * SH + rank
        self.global_row = global_row

        srcrow = global_row[src]
        core_of = dst // SH
        rank_of = global_row[dst] - core_of * SH

        self.IDXE, self.DEGT, self.K_t, self.OFF_t = [], [], [], []
        for c in range(NC):
            m = core_of == c
            r = rank_of[m]
            v = srcrow[m]
            order = np.argsort(r, kind="stable")
            r, v = r[order], v[order]
            dsorted = np.zeros(SH, dtype=np.int64)
            dsorted[:SH] = np.bincount(r, minlength=SH)
            starts = np.concatenate([[0], np.cumsum(dsorted)])
            K_t = np.zeros(T, dtype=np.int64)
            for t in range(T):
                K_t[t] = dsorted[128 * t:128 * (t + 1)].max()
            OFF = np.concatenate([[0], np.cumsum(K_t)]).astype(np.int64)
            CK = int(OFF[-1])
            idxe = np.zeros((128, CK), dtype=np.int32)
            slot_in_dst = np.arange(len(r)) - starts[r]
            tile_of = r >> 7
            p_of = r & 127
            col = OFF[tile_of] + slot_in_dst
            idxe[p_of, col] = v
            degt = np.zeros((128, T), dtype=np.float32)
            dpad = np.concatenate([dsorted, np.zeros(T * 128 - SH, np.int64)])
            degt[:, :] = dpad.reshape(T, 128).T
            self.IDXE.append(np.ascontiguousarray(idxe))
            self.DEGT.append(degt)
            self.K_t.append(K_t)
            self.OFF_t.append(OFF)
        self.CKmax = max(int(o[-1]) for o in self.OFF_t)
        self.KMAX = max(int(k.max()) for k in self.K_t)
        # pad every core's IDXE to CKmax columns so shapes match SPMD
        for c in range(NC):
            ck = self.IDXE[c].shape[1]
            if ck < self.CKmax:
                self.IDXE[c] = np.ascontiguousarray(np.pad(
                    self.IDXE[c], ((0, 0), (0, self.CKmax - ck))))

        # decode: pair i of core c -> (p=i%128, col=i//128)
        self.PS, self.PD = [], []
        pr = global_row[ep[:, 0]].reshape(NC, cfg.PPC)
        qr = global_row[ep[:, 1]].reshape(NC, cfg.PPC)
        for c in range(NC):
            ps = pr[c].reshape(cfg.PPC // 128, 128).T.astype(np.int32)
            pd = qr[c].reshape(cfg.PPC // 128, 128).T.astype(np.int32)
            self.PS.append(np.ascontiguousarray(ps))
            self.PD.append(np.ascontiguousarray(pd))

        x = np.nan_to_num(np.asarray(x, dtype=np.float32), nan=0.0,
                          posinf=0.0, neginf=0.0)
        self.xT = []
        for c in range(NC):
            xs = x[c * SH:(c + 1) * SH][self.perm[c]]
            self.xT.append(np.ascontiguousarray(xs.T.astype(FP8)))


def host_prep(x, edge_index, edge_pairs, cfg):
    return Plan(cfg, x, edge_index, edge_pairs)


def prep_weights(inp, cfg):
    f = lambda a: np.asarray(a, np.float32)
    W = {}
    # projection weights ship sharded (1/8 per core) and are reassembled on
    # device by one AllGather; blob order must match build_program's offsets.
    blob = np.concatenate([
        f(inp["Wp"]).T.astype(BF16).ravel()] + [
        f(inp[k]).T.astype(BF16).ravel()
        for k in ("Wl1", "Wr1", "Wm1", "Wm2", "Wl2", "Wr2")])
    assert blob.size == 425984
    W["WBLOB"] = [np.ascontiguousarray(
        blob[c * 53248:(c + 1) * 53248].reshape(208, 256))
        for c in range(cfg.NC)]
    W["ATT1R"] = np.ascontiguousarray(np.broadcast_to(
        f(inp["att1"]).reshape(1, -1), (128, cfg.HID))).astype(BF16)
    W["ATT2R"] = np.ascontiguousarray(np.broadcast_to(
        f(inp["att2"]).reshape(1, -1), (128, cfg.EMB))).astype(BF16)
    W["IDENT"] = np.ascontiguousarray(np.eye(128, dtype=np.float32).astype(BF16))
    W["IOTA_ROWS"] = np.ascontiguousarray(np.broadcast_to(
        np.arange(128, dtype=np.float32), (128, 128))).copy()
    alpha = 1.0 / (1.0 + np.exp(-float(f(inp["logit_alpha"]).ravel()[0])))
    temp = float(f(inp["temperature"]))
    W["A12R"] = np.ascontiguousarray(np.broadcast_to(
        np.array([alpha * temp, (1.0 - alpha) * temp], np.float32),
        (128, 2))).copy()
    return W


# ---------------------------------------------------------------------------
# device program
# ---------------------------------------------------------------------------

def build_program(plan, cfg, stage=5):
    import contextlib
    import concourse.bass as bass
    import concourse.tile as tile
    from concourse import bacc, mybir

    dt = mybir.dt
    AF = mybir.ActivationFunctionType
    OP = mybir.AluOpType
    AX = mybir.AxisListType

    NC, SH, T = cfg.NC, cfg.SH, cfg.T
    RAW, IN, HID, EMB = cfg.RAW, cfg.IN, cfg.HID, cfg.EMB
    KQ = RAW // 128
    N8 = SH * NC
    CK = plan.CKmax
    KMAX = plan.KMAX
    K_t, OFF_t = plan.K_t[0], plan.OFF_t[0]   # identical structure per core?
    EPS_LN = 1e-5
    EPS_DEN = 1e-16

    # NOTE: K_t / OFF_t differ per core. SPMD emits ONE program, so loop
    # bounds must be core-independent: use the per-tile MAX over cores.
    K_t = np.stack([plan.K_t[c] for c in range(NC)]).max(axis=0)
    OFF = np.concatenate([[0], np.cumsum(K_t)]).astype(np.int64)
    assert OFF[-1] <= CK or True
    CKU = int(OFF[-1])

    nc = bacc.Bacc("TRN2", target_bir_lowering=False, debug=False,
                   num_devices=NC)

    din = lambda name, shape, d: nc.dram_tensor(name, shape, d,
                                                kind="ExternalInput").ap()
    xT = din("xT", [RAW, SH], dt.float8e4)
    IDXE = din("IDXE", [128, CKU], dt.int32)
    DEGT = din("DEGT", [128, T], dt.float32)
    PS32 = din("PS32", [128, cfg.PPC // 128], dt.int32)
    PD32 = din("PD32", [128, cfg.PPC // 128], dt.int32)
    WBLOB = din("WBLOB", [208, 256], dt.bfloat16)
    ATT1R = din("ATT1R", [128, HID], dt.bfloat16)
    ATT2R = din("ATT2R", [128, EMB], dt.bfloat16)
    IDENT = din("IDENT", [128, 128], dt.bfloat16)
    IOTA_ROWS = din("IOTA_ROWS", [128, 128], dt.float32)
    A12R = din("A12R", [128, 2], dt.float32)

    res_out = nc.dram_tensor("res", [cfg.PPC], dt.float32,
                             kind="ExternalOutput").ap()

    rg = [list(range(NC))]
    sems = {ph: nc.alloc_semaphore(f"gsem_{ph}") for ph in ("a", "b", "d")}
    gcnt = {ph: 0 for ph in ("a", "b", "d")}

    def rows(t):
        return min(128, SH - 128 * t)

    with tile.TileContext(nc) as tc:
        ctx = contextlib.ExitStack()
        with ctx:
            cpool = ctx.enter_context(tc.tile_pool(name="consts", bufs=1))
            dpool = ctx.enter_context(tc.tile_pool(name="dram", bufs=1,
                                                   space="DRAM"))
            sstat = ctx.enter_context(tc.tile_pool(name="sstat", bufs=2))
            dps = ctx.enter_context(tc.tile_pool(name="dps", bufs=2,
                                                 space="PSUM"))

            def cload(ap, shape, d=dt.bfloat16, name=None):
                t_ = cpool.tile(shape, d, name=name)
                nc.sync.dma_start(t_[:], ap)
                return t_

            # reassemble the sharded weight blob: upload -> own DRAM slice ->
            # AllGather -> per-weight strided loads into SBUF.
            wblob_own = dpool.tile([208, 256], dt.bfloat16, name="wblob_own")
            wblob = dpool.tile([1664, 256], dt.bfloat16, name="wblob",
                               addr_space="Shared")
            wsb = cpool.tile([128, 416], dt.bfloat16, name="wsb")
            nc.sync.dma_start(
                wsb[:], bass.AP(WBLOB.tensor, 0, [[416, 128], [1, 416]]))
            nc.sync.dma_start(
                bass.AP(wblob_own.tensor, wblob_own.offset,
                        [[416, 128], [1, 416]]), wsb[:])
            nc.gpsimd.collective_compute(
                "AllGather", OP.bypass, replica_groups=rg,
                ins=[wblob_own[:].opt()], outs=[wblob[:].opt()])

            def wload(off, kq, Dout, name):
                return cload(
                    bass.AP(wblob.tensor, wblob.offset + off,
                            [[Dout, 128], [128 * Dout, kq], [1, Dout]]),
                    [128, kq, Dout], name=name)

            wpT_s = wload(0, KQ, IN, "wpT_s")
            wl1_s = wload(131072, IN // 128, HID, "wl1_s")
            wr1_s = wload(196608, IN // 128, HID, "wr1_s")
            wm1_s = wload(262144, IN // 128, HID, "wm1_s")
            wm2_s = wload(327680, HID // 128, EMB, "wm2_s")
            wl2_s = wload(360448, HID // 128, EMB, "wl2_s")
            wr2_s = wload(393216, HID // 128, EMB, "wr2_s")
            att1_s = cload(ATT1R, [128, HID], name="att1_s")
            att2_s = cload(ATT2R, [128, EMB], name="att2_s")
            ident_s = cload(IDENT, [128, 128], name="ident_s")
            iota_s = cload(IOTA_ROWS, [128, 128], dt.float32, name="iota_s")
            a12_s = cload(A12R, [128, 2], dt.float32, name="a12_s")
            deg_s = cload(DEGT, [128, T], dt.float32, name="deg_s")
            idxe_s = cload(IDXE, [128, CKU], dt.int32, name="idxe_s")

            xr1_all = cpool.tile([128, T, IN], dt.bfloat16, name="xr1_all")
            xr2_all = cpool.tile([128, T, EMB], dt.bfloat16, name="xr2_all")

            xl1_own = dpool.tile([SH, HID], dt.bfloat16, name="xl1_own")
            xl2_own = dpool.tile([SH, EMB], dt.bfloat16, name="xl2_own")
            z_own = dpool.tile([SH, 2 * EMB], dt.bfloat16, name="z_own")
            xl1_tbl = dpool.tile([N8, HID], dt.bfloat16, name="xl1_tbl",
                                 addr_space="Shared")
            xl2_tbl = dpool.tile([N8, EMB], dt.bfloat16, name="xl2_tbl",
                                 addr_space="Shared")
            z_tbl = dpool.tile([N8, 2 * EMB], dt.bfloat16, name="z_tbl",
                               addr_space="Shared")

            # -------- helpers --------
            def ln_relu(src_t, n, D, out_bf):
                """out = relu(layer_norm(src)); scale-invariant in src."""
                sm = sstat.tile([128, 1], dt.float32, name="sm", tag="sm")
                nc.vector.tensor_reduce(sm[:n], src_t[:n, :D], axis=AX.X,
                                        op=OP.add)
                scr = sstat.tile([128, 256], dt.float32, name="scr", tag="scr")
                sq = sstat.tile([128, 1], dt.float32, name="sq", tag="sq")
                nc.scalar.activation(scr[:n, :D], src_t[:n, :D], AF.Square,
                                     accum_out=sq[:n])
                msq = sstat.tile([128, 1], dt.float32, name="msq", tag="msq")
                nc.vector.scalar_tensor_tensor(out=msq[:n], in0=sm[:n],
                                               scalar=1.0 / (D * D),
                                               in1=sm[:n], op0=OP.mult,
                                               op1=OP.mult)
                var = sstat.tile([128, 1], dt.float32, name="var", tag="var")
                nc.vector.scalar_tensor_tensor(out=var[:n], in0=sq[:n],
                                               scalar=1.0 / D, in1=msq[:n],
                                               op0=OP.mult, op1=OP.subtract)
                veps = sstat.tile([128, 1], dt.float32, name="veps", tag="veps")
                nc.vector.tensor_scalar(out=veps[:n], in0=var[:n],
                                        scalar1=EPS_LN, scalar2=None,
                                        op0=OP.add)
                rinv = sstat.tile([128, 1], dt.float32, name="rinv", tag="rinv")
                nc.vector.reciprocal(rinv[:n], veps[:n])
                rstd = sstat.tile([128, 1], dt.float32, name="rstd", tag="rstd")
                nc.scalar.activation(rstd[:n], rinv[:n], AF.Sqrt)
                nb = sstat.tile([128, 1], dt.float32, name="nb", tag="nb")
                nc.vector.scalar_tensor_tensor(out=nb[:n], in0=sm[:n],
                                               scalar=-1.0 / D, in1=rstd[:n],
                                               op0=OP.mult, op1=OP.mult)
                nc.scalar.activation(out_bf[:n, :D], src_t[:n, :D], AF.Relu,
                                     bias=nb[:n], scale=rstd[:n])

            def transpose_to(pool, src_bf, n, D, name):
                out = pool.tile([128, D // 128, 128], dt.bfloat16, name=name,
                                tag=name, padded_shape=[128, 2, 128])
                for b in range(D // 128):
                    tp = dps.tile([128, 128], dt.bfloat16, name=name + "_ps",
                                  tag="tp", space="PSUM", bufs=1)
                    nc.tensor.transpose(tp[:, :n],
                                        src_bf[:n, 128 * b:128 * (b + 1)],
                                        ident_s[:n, :n])
                    nc.scalar.copy(out[:, b, :n], tp[:, :n])
                return out

            def proj(inT, n, wT, Dout, name, kchunks):
                ps_t = dps.tile([128, 256], dt.float32, name=name + "_ps",
                                tag="proj", space="PSUM", bufs=1)
                for q in range(kchunks):
                    nc.tensor.matmul(out=ps_t[:n, :Dout], lhsT=inT[:, q, :n],
                                     rhs=wT[:, q, :], start=(q == 0),
                                     stop=(q == kchunks - 1),
                                     skip_group_check=True)
                return ps_t

            # ================= dense phase =================
            with tc.tile_pool(name="dsb", bufs=2) as dsb:
                for t in range(T):
                    n = rows(t)
                    xt = dsb.tile([128, KQ, 128], dt.float8e4, name="xt")
                    nc.sync.dma_start(
                        xt[:, :, :n],
                        xT.rearrange("(q p) m -> p q m", p=128)[:, :, 128 * t:128 * t + n])
                    xp_ps = proj(xt, n, wpT_s, IN, "xp", KQ)
                    xp = dsb.tile([128, IN], dt.bfloat16, name="xp")
                    ln_relu(xp_ps, n, IN, xp)
                    xpT = transpose_to(dsb, xp, n, IN, "xpT")

                    xl1_ps = proj(xpT, n, wl1_s, HID, "xl1", IN // 128)
                    xl1_bf = dsb.tile([128, HID], dt.bfloat16, name="xl1_bf")
                    nc.scalar.copy(xl1_bf[:n, :], xl1_ps[:n, :HID])
                    nc.sync.dma_start(xl1_own[128 * t:128 * t + n, :],
                                      xl1_bf[:n, :])

                    xr1_ps = proj(xpT, n, wr1_s, HID, "xr1", IN // 128)
                    nc.vector.tensor_copy(xr1_all[:n, t, :], xr1_ps[:n, :HID])

                    m1_ps = proj(xpT, n, wm1_s, HID, "m1", IN // 128)
                    m1 = dsb.tile([128, HID], dt.bfloat16, name="m1")
                    ln_relu(m1_ps, n, HID, m1)
                    m1T = transpose_to(dsb, m1, n, HID, "m1T")
                    zf_ps = proj(m1T, n, wm2_s, EMB, "zf", HID // 128)
                    zf_bf = dsb.tile([128, EMB], dt.bfloat16, name="zf_bf")
                    nc.vector.tensor_copy(zf_bf[:n, :], zf_ps[:n, :EMB])
                    nc.sync.dma_start(z_own[128 * t:128 * t + n, EMB:],
                                      zf_bf[:n, :])

            if stage >= 2:
                nc.gpsimd.collective_compute(
                    "AllGather", OP.bypass, replica_groups=rg,
                    ins=[xl1_own[:].opt()], outs=[xl1_tbl[:].opt()])

            # ================= edge phase (CSR wide ops) =================
            def edge_tile(pools, t, xr_all, tbl, D, H, att_s, out_cb, suf):
                esb = pools["esb"]
                n = rows(t)
                Kt = int(K_t[t])
                c0 = int(OFF[t])
                DH = D // H
                psem = sems[suf]

                xg = esb.tile([128, Kt * D], dt.bfloat16, name=f"xg{suf}",
                              tag=f"xg{suf}", padded_shape=[128, KMAX * D])
                for k in range(Kt):
                    nc.gpsimd.indirect_dma_start(
                        out=xg[:, k * D:(k + 1) * D], out_offset=None,
                        in_=tbl[:],
                        in_offset=bass.IndirectOffsetOnAxis(
                            ap=idxe_s[:, c0 + k:c0 + k + 1], axis=0),
                    ).then_inc(psem, 16)
                gcnt[suf] += Kt
                nc.vector.tensor_copy(xg[:1, :1], xg[:1, :1])._wait_ge(
                    psem, 16 * gcnt[suf])

                # pad mask: 1.0 where k >= deg (padded slot)
                mpad = esb.tile([128, Kt], dt.float32, name=f"mp{suf}",
                                tag=f"mp{suf}", padded_shape=[128, KMAX])
                nc.vector.tensor_tensor(
                    out=mpad[:, :Kt], in0=iota_s[:, :Kt],
                    in1=bass.AP(deg_s.tensor, deg_s.offset + t,
                                [list(deg_s.ap[0]), [0, Kt]]),
                    op=OP.is_ge)

                # e = lrelu(xg + xr[dst]) ; score = <e, att> per head
                e_t = esb.tile([128, Kt * D], dt.bfloat16, name=f"e{suf}",
                               tag=f"e{suf}", padded_shape=[128, KMAX * D])
                xr_b = bass.AP(xr_all.tensor, xr_all.offset + t * D,
                               [list(xr_all.ap[0]), [0, Kt], [1, D]])
                nc.vector.tensor_tensor(out=e_t[:, :Kt * D],
                                        in0=xg[:, :Kt * D], in1=xr_b,
                                        op=OP.add)
                e2_t = esb.tile([128, Kt * D], dt.bfloat16, name=f"e2{suf}",
                                tag=f"e2{suf}", padded_shape=[128, KMAX * D])
                nc.vector.scalar_tensor_tensor(
                    out=e2_t[:, :Kt * D], in0=e_t[:, :Kt * D], scalar=0.2,
                    in1=e_t[:, :Kt * D], op0=OP.mult, op1=OP.max)
                att_b = bass.AP(att_s.tensor, att_s.offset,
                                [list(att_s.ap[0]), [0, Kt], [1, D]])
                sm_t = esb.tile([128, Kt * D], dt.bfloat16, name=f"smt{suf}",
                                tag=f"e{suf}", padded_shape=[128, KMAX * D])
                nc.vector.tensor_tensor(out=sm_t[:, :Kt * D],
                                        in0=e2_t[:, :Kt * D], in1=att_b,
                                        op=OP.mult)
                sc = esb.tile([128, Kt * H], dt.float32, name=f"sc{suf}",
                              tag=f"sc{suf}", padded_shape=[128, KMAX * H])
                nc.vector.tensor_reduce(
                    out=sc[:, :Kt * H],
                    in_=bass.AP(sm_t.tensor, sm_t.offset,
                                [list(sm_t.ap[0]), [DH, Kt * H], [1, DH]]),
                    axis=AX.X, op=OP.add)
                # mask pad slots: score += -100 * mpad   (broadcast over heads)
                mpad_b = bass.AP(mpad.tensor, mpad.offset,
                                 [list(mpad.ap[0]), [1, Kt], [0, H]])
                nc.vector.scalar_tensor_tensor(
                    out=sc[:, :Kt * H], in0=mpad_b, scalar=-100.0,
                    in1=sc[:, :Kt * H], op0=OP.mult, op1=OP.add)
                ex = esb.tile([128, Kt * H], dt.float32, name=f"ex{suf}",
                              tag=f"ex{suf}", padded_shape=[128, KMAX * H])
                nc.scalar.activation(ex[:, :Kt * H], sc[:, :Kt * H], AF.Exp)
                den = sstat.tile([128, 8], dt.float32, name=f"den{suf}",
                                 tag=f"den{suf}")
                nc.vector.tensor_reduce(
                    out=den[:, :H],
                    in_=bass.AP(ex.tensor, ex.offset,
                                [list(ex.ap[0]), [1, H], [H, Kt]]),
                    axis=AX.X, op=OP.add)
                dene = sstat.tile([128, 8], dt.float32, name=f"dene{suf}",
                                  tag=f"dene{suf}")
                nc.vector.tensor_scalar(out=dene[:, :H], in0=den[:, :H],
                                        scalar1=EPS_DEN, scalar2=None,
                                        op0=OP.add)
                rec = sstat.tile([128, 8], dt.float32, name=f"rec{suf}",
                                 tag=f"rec{suf}")
                nc.vector.reciprocal(rec[:, :H], dene[:, :H])
                alp = esb.tile([128, Kt * H], dt.bfloat16, name=f"al{suf}",
                               tag=f"al{suf}", padded_shape=[128, KMAX * H])
                rec_b = bass.AP(rec.tensor, rec.offset,
                                [list(rec.ap[0]), [0, Kt], [1, H]])
                nc.vector.tensor_tensor(out=alp[:, :Kt * H],
                                        in0=ex[:, :Kt * H], in1=rec_b,
                                        op=OP.mult)
                # w = xg * alpha ; out = sum_k w
                w_t = esb.tile([128, Kt * D], dt.bfloat16, name=f"w{suf}",
                               tag=f"e2{suf}", padded_shape=[128, KMAX * D])
                alp_b = bass.AP(alp.tensor, alp.offset,
                                [list(alp.ap[0]), [H, Kt], [1, H], [0, DH]])
                nc.vector.tensor_tensor(out=w_t[:, :Kt * D],
                                        in0=xg[:, :Kt * D], in1=alp_b,
                                        op=OP.mult)
                outf = esb.tile([128, D], dt.float32, name=f"o{suf}",
                                tag=f"o{suf}")
                nc.vector.tensor_reduce(
                    out=outf[:, :D],
                    in_=bass.AP(w_t.tensor, w_t.offset,
                                [list(w_t.ap[0]), [1, D], [D, Kt]]),
                    axis=AX.X, op=OP.add)
                out_cb(pools, outf, n, t)

            def l1_out(pools, outf, n, t):
                esb = pools["esb"]
                h_bf = esb.tile([128, HID], dt.bfloat16, name="h_bf",
                                tag="h_bf")
                ln_relu(outf, n, HID, h_bf)
                hT = transpose_to(esb, h_bf, n, HID, "hT")
                xl2_ps = proj(hT, n, wl2_s, EMB, "xl2", HID // 128)
                xl2_bf = esb.tile([128, EMB], dt.bfloat16, name="xl2_bf",
                                  tag="xl2_bf")
                nc.scalar.copy(xl2_bf[:n, :], xl2_ps[:n, :EMB])
                nc.sync.dma_start(xl2_own[128 * t:128 * t + n, :],
                                  xl2_bf[:n, :])
                xr2_ps = proj(hT, n, wr2_s, EMB, "xr2", HID // 128)
                nc.vector.tensor_copy(xr2_all[:n, t, :], xr2_ps[:n, :EMB])

            def l2_out(pools, outf, n, t):
                esb = pools["esb"]
                zg = esb.tile([128, EMB], dt.bfloat16, name="zg", tag="zg")
                nc.vector.tensor_copy(zg[:n, :], outf[:n, :EMB])
                nc.sync.dma_start(z_own[128 * t:128 * t + n, :EMB], zg[:n, :])

            if stage >= 3:
                with tc.tile_pool(name="esb_a", bufs=1) as esb_a:
                    pools = {"esb": esb_a}
                    for t in range(T):
                        edge_tile(pools, t, xr1_all, xl1_tbl, HID, 4, att1_s,
                                  l1_out, "a")

            if stage >= 4:
                nc.gpsimd.collective_compute(
                    "AllGather", OP.bypass, replica_groups=rg,
                    ins=[xl2_own[:].opt()], outs=[xl2_tbl[:].opt()])

                with tc.tile_pool(name="esb_b", bufs=1) as esb_b:
                    pools = {"esb": esb_b}
                    for t in range(T):
                        edge_tile(pools, t, xr2_all, xl2_tbl, EMB, 1, att2_s,
                                  l2_out, "b")

                nc.gpsimd.collective_compute(
                    "AllGather", OP.bypass, replica_groups=rg,
                    ins=[z_own[:].opt()], outs=[z_tbl[:].opt()])

            # ================= decode =================
            D2 = 2 * EMB
            NCOL = cfg.PPC // 128          # 256
            CC = 32                        # columns per chunk
            res_sb = cpool.tile([128, NCOL], dt.float32, name="res_sb")
            if stage < 5:
                nc.vector.memset(res_sb[:], 0.0)
            with tc.tile_pool(name="dec", bufs=1) as dec:
                if stage >= 5:
                    pi_t = cpool.tile([128, NCOL], dt.int32, name="pi")
                    nc.sync.dma_start(pi_t[:], PS32)
                    pj_t = cpool.tile([128, NCOL], dt.int32, name="pj")
                    nc.sync.dma_start(pj_t[:], PD32)
                for ch in range(NCOL // CC if stage >= 5 else 0):
                    o = ch * CC
                    za = dec.tile([128, CC * D2], dt.bfloat16, name="za",
                                  tag="za")
                    zb = dec.tile([128, CC * D2], dt.bfloat16, name="zb",
                                  tag="zb")
                    for j in range(CC):
                        nc.gpsimd.indirect_dma_start(
                            out=za[:, j * D2:(j + 1) * D2], out_offset=None,
                            in_=z_tbl[:],
                            in_offset=bass.IndirectOffsetOnAxis(
                                ap=pi_t[:, o + j:o + j + 1], axis=0),
                        ).then_inc(sems["d"], 16)
                        nc.gpsimd.indirect_dma_start(
                            out=zb[:, j * D2:(j + 1) * D2], out_offset=None,
                            in_=z_tbl[:],
                            in_offset=bass.IndirectOffsetOnAxis(
                                ap=pj_t[:, o + j:o + j + 1], axis=0),
                        ).then_inc(sems["d"], 16)
                    gcnt["d"] += 2 * CC
                    nc.vector.tensor_copy(za[:1, :1], za[:1, :1])._wait_ge(
                        sems["d"], 16 * gcnt["d"])
                    nc.vector.tensor_copy(zb[:1, :1], zb[:1, :1])._wait_ge(
                        sems["d"], 16 * gcnt["d"])

                    prod = dec.tile([128, CC * D2], dt.float32, name="prod",
                                    tag="prod")
                    view = lambda t_: bass.AP(
                        t_.tensor, t_.offset,
                        [list(t_.ap[0]), [EMB, CC * 2], [1, EMB]])
                    dots = dec.tile([128, CC * 2], dt.float32, name="dots",
                                    tag="dots")
                    nc.vector.tensor_tensor(out=prod[:], in0=za[:], in1=zb[:],
                                            op=OP.mult)
                    nc.vector.tensor_reduce(out=dots[:], in_=view(prod),
                                            axis=AX.X, op=OP.add)
                    sqa = dec.tile([128, CC * 2], dt.float32, name="sqa",
                                   tag="sqa")
                    nc.vector.tensor_tensor(out=prod[:], in0=za[:], in1=za[:],
                                            op=OP.mult)
                    nc.vector.tensor_reduce(out=sqa[:], in_=view(prod),
                                            axis=AX.X, op=OP.add)
                    sqb = dec.tile([128, CC * 2], dt.float32, name="sqb",
                                   tag="sqb")
                    nc.vector.tensor_tensor(out=prod[:], in0=zb[:], in1=zb[:],
                                            op=OP.mult)
                    nc.vector.tensor_reduce(out=sqb[:], in_=view(prod),
                                            axis=AX.X, op=OP.add)
                    nn_ = dec.tile([128, CC * 2], dt.float32, name="nn_",
                                   tag="nn_")
                    nc.vector.tensor_tensor(out=nn_[:], in0=sqa[:],
                                            in1=sqb[:], op=OP.mult)
                    rin = dec.tile([128, CC * 2], dt.float32, name="rin",
                                   tag="rin")
                    nc.vector.reciprocal(rin[:], nn_[:])
                    rsq = dec.tile([128, CC * 2], dt.float32, name="rsq",
                                   tag="rsq")
                    nc.scalar.activation(rsq[:], rin[:], AF.Sqrt)
                    cosv = dec.tile([128, CC * 2], dt.float32, name="cosv",
                                    tag="cosv")
                    nc.vector.tensor_tensor(out=cosv[:], in0=dots[:],
                                            in1=rsq[:], op=OP.mult)
                    wz = dec.tile([128, CC * 2], dt.float32, name="wz",
                                  tag="wz")
                    a12b = bass.AP(a12_s.tensor, a12_s.offset,
                                   [list(a12_s.ap[0]), [0, CC], [1, 2]])
                    nc.vector.tensor_tensor(out=wz[:], in0=cosv[:], in1=a12b,
                                            op=OP.mult)
                    nc.vector.tensor_reduce(
                        out=res_sb[:, o:o + CC],
                        in_=bass.AP(wz.tensor, wz.offset,
                                    [list(wz.ap[0]), [2, CC], [1, 2]]),
                        axis=AX.X, op=OP.add)

            nc.sync.dma_start(res_out.rearrange("(a b) -> b a", b=128),
                              res_sb[:])

    nc.compile()
    return nc


# ---------------------------------------------------------------------------
# entry point
# ---------------------------------------------------------------------------

def make_in_maps(plan, W, cfg):
    in_maps = []
    CKU = None
    for c in range(cfg.NC):
        m = {"xT": plan.xT[c], "DEGT": plan.DEGT[c],
             "PS32": plan.PS[c], "PD32": plan.PD[c],
             "WBLOB": W["WBLOB"][c]}
        for k in ("ATT1R", "ATT2R", "IDENT", "IOTA_ROWS", "A12R"):
            m[k] = W[k]
        in_maps.append(m)
    return in_maps


def finish_in_maps(in_maps, plan, cfg, nc):
    """Re-pack IDXE per core to the unified per-tile offsets of the program."""
    K_t = np.stack([plan.K_t[c] for c in range(cfg.NC)]).max(axis=0)
    OFF = np.concatenate([[0], np.cumsum(K_t)]).astype(np.int64)
    CKU = int(OFF[-1])
    for c in range(cfg.NC):
        idxe = np.zeros((128, CKU), dtype=np.int32)
        for t in range(cfg.T):
            kc = int(plan.K_t[c][t])
            oc = int(plan.OFF_t[c][t])
            idxe[:, int(OFF[t]):int(OFF[t]) + kc] = \
                plan.IDXE[c][:, oc:oc + kc]
        in_maps[c]["IDXE"] = idxe
    return in_maps


def kernel(**inputs):
    cfg = CFG
    plan = host_prep(inputs["x"], inputs["edge_index"],
                     inputs["edge_pairs"], cfg)
    W = prep_weights(inputs, cfg)
    nc = build_program(plan, cfg)
    from concourse.bass_utils import run_bass_kernel_spmd
    in_maps = finish_in_maps(make_in_maps(plan, W, cfg), plan, cfg, nc)
    res = run_bass_kernel_spmd(nc, in_maps, core_ids=list(range(cfg.NC)))
    out = np.concatenate([np.asarray(res.results[c]["res"])
                          for c in range(cfg.NC)])
    return out.astype(np.float32)


# revision 10
# speedup vs baseline: 6.0538x; 1.1606x over previous
"""Trainium2 Bass kernel for nn_DualSignalLinkPredictorC (2-layer GATv2 + MLP
link predictor), distributed over 8 NeuronCores.

v2: degree-sorted CSR layout. The backend executes ~1 instruction per 45us
regardless of width, so the design minimizes instruction count:
  - dst nodes on partitions, neighbors padded along the free dim (K_t = max
    in-degree per 128-node tile). Per-core nodes are permuted by descending
    degree so K_t tracks the local mean (~18) instead of the global max.
  - per-edge attention, segment softmax, and aggregation are wide DVE ops
    over [128, K_t*D] tiles; no one-hot matmuls, no PE transposes in the
    edge phase. Gathers are one indirect DMA per neighbor column.
  - single AllGather per table (no chunking; int32 row indices).
  - x is uploaded in fp8 (e4m3) to halve the dominant input transfer.
"""

import numpy as np
import ml_dtypes

BF16 = ml_dtypes.bfloat16
FP8 = ml_dtypes.float8_e4m3fn


class Cfg:
    def __init__(self, N=100000, E=1600000, NPAIRS=262144, NC=8,
                 RAW=512, IN=256, HID=256, EMB=128):
        self.N, self.E, self.NPAIRS, self.NC = N, E, NPAIRS, NC
        self.RAW, self.IN, self.HID, self.EMB = RAW, IN, HID, EMB
        assert N % NC == 0
        self.SH = N // NC
        self.T = (self.SH + 127) // 128
        self.PPC = NPAIRS // NC
        assert self.PPC % 128 == 0


CFG = Cfg()


class Plan:
    """Degree-sorted CSR neighbor plan + decode indices, per core."""

    def __init__(self, cfg, x, edge_index, edge_pairs):
        NC, SH, T = cfg.NC, cfg.SH, cfg.T
        ei = np.asarray(edge_index, dtype=np.int64)
        ep = np.asarray(edge_pairs, dtype=np.int64)
        loops = np.arange(cfg.N, dtype=np.int64)
        src = np.concatenate([ei[0], loops])
        dst = np.concatenate([ei[1], loops])

        deg = np.bincount(dst, minlength=cfg.N)
        # per-core descending-degree permutation; global_row maps node id ->
        # row in the AllGathered tables (core-major, rank within core).
        self.perm = []           # per core: rank -> local node
        global_row = np.empty(cfg.N, dtype=np.int64)
        for c in range(NC):
            dloc = deg[c * SH:(c + 1) * SH]
            p = np.argsort(-dloc, kind="stable")
            self.perm.append(p)
            rank = np.empty(SH, dtype=np.int64)
            rank[p] = np.arange(SH)
            global_row[c * SH:(c + 1) * SH] = c * SH + rank
        self.global_row = global_row

        srcrow = global_row[src]
        core_of = dst // SH
        rank_of = global_row[dst] - core_of * SH

        self.IDXE, self.DEGT, self.K_t, self.OFF_t = [], [], [], []
        for c in range(NC):
            m = core_of == c
            r = rank_of[m]
            v = srcrow[m]
            order = np.argsort(r, kind="stable")
            r, v = r[order], v[order]
            dsorted = np.zeros(SH, dtype=np.int64)
            dsorted[:SH] = np.bincount(r, minlength=SH)
            starts = np.concatenate([[0], np.cumsum(dsorted)])
            K_t = np.zeros(T, dtype=np.int64)
            for t in range(T):
                K_t[t] = dsorted[128 * t:128 * (t + 1)].max()
            OFF = np.concatenate([[0], np.cumsum(K_t)]).astype(np.int64)
            CK = int(OFF[-1])
            idxe = np.zeros((128, CK), dtype=np.int32)
            slot_in_dst = np.arange(len(r)) - starts[r]
            tile_of = r >> 7
            p_of = r & 127
            col = OFF[tile_of] + slot_in_dst
            idxe[p_of, col] = v
            degt = np.zeros((128, T), dtype=np.float32)
            dpad = np.concatenate([dsorted, np.zeros(T * 128 - SH, np.int64)])
            degt[:, :] = dpad.reshape(T, 128).T
            self.IDXE.append(np.ascontiguousarray(idxe))
            self.DEGT.append(degt)
            self.K_t.append(K_t)
            self.OFF_t.append(OFF)
        self.CKmax = max(int(o[-1]) for o in self.OFF_t)
        self.KMAX = max(int(k.max()) for k in self.K_t)
        # pad every core's IDXE to CKmax columns so shapes match SPMD
        for c in range(NC):
            ck = self.IDXE[c].shape[1]
            if ck < self.CKmax:
                self.IDXE[c] = np.ascontiguousarray(np.pad(
                    self.IDXE[c], ((0, 0), (0, self.CKmax - ck))))

        # decode: pair i of core c -> (p=i%128, col=i//128)
        self.PS, self.PD = [], []
        pr = global_row[ep[:, 0]].reshape(NC, cfg.PPC)
        qr = global_row[ep[:, 1]].reshape(NC, cfg.PPC)
        for c in range(NC):
            ps = pr[c].reshape(cfg.PPC // 128, 128).T.astype(np.int32)
            pd = qr[c].reshape(cfg.PPC // 128, 128).T.astype(np.int32)
            self.PS.append(np.ascontiguousarray(ps))
            self.PD.append(np.ascontiguousarray(pd))

        x = np.nan_to_num(np.asarray(x, dtype=np.float32), nan=0.0,
                          posinf=0.0, neginf=0.0)
        self.xT = []
        for c in range(NC):
            xs = x[c * SH:(c + 1) * SH][self.perm[c]]
            self.xT.append(np.ascontiguousarray(xs.T.astype(FP8)))


def host_prep(x, edge_index, edge_pairs, cfg):
    return Plan(cfg, x, edge_index, edge_pairs)


def prep_weights(inp, cfg):
    f = lambda a: np.asarray(a, np.float32)
    W = {}
    # projection weights ship sharded (1/8 per core) and are reassembled on
    # device by one AllGather; blob order must match build_program's offsets.
    blob = np.concatenate([
        f(inp["Wp"]).T.astype(BF16).ravel()] + [
        f(inp[k]).T.astype(BF16).ravel()
        for k in ("Wl1", "Wr1", "Wm1", "Wm2", "Wl2", "Wr2")])
    assert blob.size == 425984
    W["WBLOB"] = [np.ascontiguousarray(
        blob[c * 53248:(c + 1) * 53248].reshape(208, 256))
        for c in range(cfg.NC)]
    W["ATT1R"] = np.ascontiguousarray(np.broadcast_to(
        f(inp["att1"]).reshape(1, -1), (128, cfg.HID))).astype(BF16)
    W["ATT2R"] = np.ascontiguousarray(np.broadcast_to(
        f(inp["att2"]).reshape(1, -1), (128, cfg.EMB))).astype(BF16)
    W["IDENT"] = np.ascontiguousarray(np.eye(128, dtype=np.float32).astype(BF16))
    W["IOTA_ROWS"] = np.ascontiguousarray(np.broadcast_to(
        np.arange(128, dtype=np.float32), (128, 128))).copy()
    alpha = 1.0 / (1.0 + np.exp(-float(f(inp["logit_alpha"]).ravel()[0])))
    temp = float(f(inp["temperature"]))
    W["A12R"] = np.ascontiguousarray(np.broadcast_to(
        np.array([alpha * temp, (1.0 - alpha) * temp], np.float32),
        (128, 2))).copy()
    return W


# ---------------------------------------------------------------------------
# device program
# ---------------------------------------------------------------------------

def build_program(plan, cfg, stage=5):
    import contextlib
    import concourse.bass as bass
    import concourse.tile as tile
    from concourse import bacc, mybir

    dt = mybir.dt
    AF = mybir.ActivationFunctionType
    OP = mybir.AluOpType
    AX = mybir.AxisListType

    NC, SH, T = cfg.NC, cfg.SH, cfg.T
    RAW, IN, HID, EMB = cfg.RAW, cfg.IN, cfg.HID, cfg.EMB
    KQ = RAW // 128
    N8 = SH * NC
    CK = plan.CKmax
    KMAX = plan.KMAX
    K_t, OFF_t = plan.K_t[0], plan.OFF_t[0]   # identical structure per core?
    EPS_LN = 1e-5
    EPS_DEN = 1e-16

    # NOTE: K_t / OFF_t differ per core. SPMD emits ONE program, so loop
    # bounds must be core-independent: use the per-tile MAX over cores.
    K_t = np.stack([plan.K_t[c] for c in range(NC)]).max(axis=0)
    OFF = np.concatenate([[0], np.cumsum(K_t)]).astype(np.int64)
    assert OFF[-1] <= CK or True
    CKU = int(OFF[-1])

    nc = bacc.Bacc("TRN2", target_bir_lowering=False, debug=False,
                   num_devices=NC)

    din = lambda name, shape, d: nc.dram_tensor(name, shape, d,
                                                kind="ExternalInput").ap()
    xT = din("xT", [RAW, SH], dt.float8e4)
    IDXE = din("IDXE", [128, CKU], dt.int32)
    DEGT = din("DEGT", [128, T], dt.float32)
    PS32 = din("PS32", [128, cfg.PPC // 128], dt.int32)
    PD32 = din("PD32", [128, cfg.PPC // 128], dt.int32)
    WBLOB = din("WBLOB", [208, 256], dt.bfloat16)
    ATT1R = din("ATT1R", [128, HID], dt.bfloat16)
    ATT2R = din("ATT2R", [128, EMB], dt.bfloat16)
    IDENT = din("IDENT", [128, 128], dt.bfloat16)
    IOTA_ROWS = din("IOTA_ROWS", [128, 128], dt.float32)
    A12R = din("A12R", [128, 2], dt.float32)

    res_out = nc.dram_tensor("res", [cfg.PPC], dt.float32,
                             kind="ExternalOutput").ap()

    rg = [list(range(NC))]
    sems = {ph: nc.alloc_semaphore(f"gsem_{ph}") for ph in ("a", "b", "d")}
    gcnt = {ph: 0 for ph in ("a", "b", "d")}

    def rows(t):
        return min(128, SH - 128 * t)

    with tile.TileContext(nc) as tc:
        ctx = contextlib.ExitStack()
        with ctx:
            cpool = ctx.enter_context(tc.tile_pool(name="consts", bufs=1))
            dpool = ctx.enter_context(tc.tile_pool(name="dram", bufs=1,
                                                   space="DRAM"))
            sstat = ctx.enter_context(tc.tile_pool(name="sstat", bufs=2))
            dps = ctx.enter_context(tc.tile_pool(name="dps", bufs=2,
                                                 space="PSUM"))

            def cload(ap, shape, d=dt.bfloat16, name=None):
                t_ = cpool.tile(shape, d, name=name)
                nc.sync.dma_start(t_[:], ap)
                return t_

            # reassemble the sharded weight blob: upload -> own DRAM slice ->
            # AllGather -> per-weight strided loads into SBUF.
            wblob_own = dpool.tile([208, 256], dt.bfloat16, name="wblob_own")
            wblob = dpool.tile([1664, 256], dt.bfloat16, name="wblob",
                               addr_space="Shared")
            wsb = cpool.tile([128, 416], dt.bfloat16, name="wsb")
            nc.sync.dma_start(
                wsb[:], bass.AP(WBLOB.tensor, 0, [[416, 128], [1, 416]]))
            nc.sync.dma_start(
                bass.AP(wblob_own.tensor, wblob_own.offset,
                        [[416, 128], [1, 416]]), wsb[:])
            nc.gpsimd.collective_compute(
                "AllGather", OP.bypass, replica_groups=rg,
                ins=[wblob_own[:].opt()], outs=[wblob[:].opt()])

            def wload(off, kq, Dout, name):
                return cload(
                    bass.AP(wblob.tensor, wblob.offset + off,
                            [[Dout, 128], [128 * Dout, kq], [1, Dout]]),
                    [128, kq, Dout], name=name)

            wpT_s = wload(0, KQ, IN, "wpT_s")
            wl1_s = wload(131072, IN // 128, HID, "wl1_s")
            wr1_s = wload(196608, IN // 128, HID, "wr1_s")
            wm1_s = wload(262144, IN // 128, HID, "wm1_s")
            wm2_s = wload(327680, HID // 128, EMB, "wm2_s")
            wl2_s = wload(360448, HID // 128, EMB, "wl2_s")
            wr2_s = wload(393216, HID // 128, EMB, "wr2_s")
            att1_s = cload(ATT1R, [128, HID], name="att1_s")
            att2_s = cload(ATT2R, [128, EMB], name="att2_s")
            ident_s = cload(IDENT, [128, 128], name="ident_s")
            iota_s = cload(IOTA_ROWS, [128, 128], dt.float32, name="iota_s")
            a12_s = cload(A12R, [128, 2], dt.float32, name="a12_s")
            deg_s = cload(DEGT, [128, T], dt.float32, name="deg_s")
            idxe_s = cload(IDXE, [128, CKU], dt.int32, name="idxe_s")

            xr1_all = cpool.tile([128, T, IN], dt.bfloat16, name="xr1_all")
            xr2_all = cpool.tile([128, T, EMB], dt.bfloat16, name="xr2_all")

            xl1_own = dpool.tile([SH, HID], dt.bfloat16, name="xl1_own")
            xl2_own = dpool.tile([SH, EMB], dt.bfloat16, name="xl2_own")
            z_own = dpool.tile([SH, 2 * EMB], dt.bfloat16, name="z_own")
            xl1_tbl = dpool.tile([N8, HID], dt.bfloat16, name="xl1_tbl",
                                 addr_space="Shared")
            xl2_tbl = dpool.tile([N8, EMB], dt.bfloat16, name="xl2_tbl",
                                 addr_space="Shared")
            z_tbl = dpool.tile([N8, 2 * EMB], dt.bfloat16, name="z_tbl",
                               addr_space="Shared")

            # -------- helpers --------
            def ln_relu(src_t, n, D, out_bf):
                """out = relu(layer_norm(src)); scale-invariant in src."""
                sm = sstat.tile([128, 1], dt.float32, name="sm", tag="sm")
                nc.vector.tensor_reduce(sm[:n], src_t[:n, :D], axis=AX.X,
                                        op=OP.add)
                scr = sstat.tile([128, 256], dt.float32, name="scr", tag="scr")
                sq = sstat.tile([128, 1], dt.float32, name="sq", tag="sq")
                nc.scalar.activation(scr[:n, :D], src_t[:n, :D], AF.Square,
                                     accum_out=sq[:n])
                msq = sstat.tile([128, 1], dt.float32, name="msq", tag="msq")
                nc.vector.scalar_tensor_tensor(out=msq[:n], in0=sm[:n],
                                               scalar=1.0 / (D * D),
                                               in1=sm[:n], op0=OP.mult,
                                               op1=OP.mult)
                var = sstat.tile([128, 1], dt.float32, name="var", tag="var")
                nc.vector.scalar_tensor_tensor(out=var[:n], in0=sq[:n],
                                               scalar=1.0 / D, in1=msq[:n],
                                               op0=OP.mult, op1=OP.subtract)
                veps = sstat.tile([128, 1], dt.float32, name="veps", tag="veps")
                nc.vector.tensor_scalar(out=veps[:n], in0=var[:n],
                                        scalar1=EPS_LN, scalar2=None,
                                        op0=OP.add)
                rinv = sstat.tile([128, 1], dt.float32, name="rinv", tag="rinv")
                nc.vector.reciprocal(rinv[:n], veps[:n])
                rstd = sstat.tile([128, 1], dt.float32, name="rstd", tag="rstd")
                nc.scalar.activation(rstd[:n], rinv[:n], AF.Sqrt)
                nb = sstat.tile([128, 1], dt.float32, name="nb", tag="nb")
                nc.vector.scalar_tensor_tensor(out=nb[:n], in0=sm[:n],
                                               scalar=-1.0 / D, in1=rstd[:n],
                                               op0=OP.mult, op1=OP.mult)
                nc.scalar.activation(out_bf[:n, :D], src_t[:n, :D], AF.Relu,
                                     bias=nb[:n], scale=rstd[:n])

            def transpose_to(pool, src_bf, n, D, name):
                out = pool.tile([128, D // 128, 128], dt.bfloat16, name=name,
                                tag=name, padded_shape=[128, 2, 128])
                for b in range(D // 128):
                    tp = dps.tile([128, 128], dt.bfloat16, name=name + "_ps",
                                  tag="tp", space="PSUM", bufs=1)
                    nc.tensor.transpose(tp[:, :n],
                                        src_bf[:n, 128 * b:128 * (b + 1)],
                                        ident_s[:n, :n])
                    nc.scalar.copy(out[:, b, :n], tp[:, :n])
                return out

            def proj(inT, n, wT, Dout, name, kchunks):
                ps_t = dps.tile([128, 256], dt.float32, name=name + "_ps",
                                tag="proj", space="PSUM", bufs=1)
                for q in range(kchunks):
                    nc.tensor.matmul(out=ps_t[:n, :Dout], lhsT=inT[:, q, :n],
                                     rhs=wT[:, q, :], start=(q == 0),
                                     stop=(q == kchunks - 1),
                                     skip_group_check=True)
                return ps_t

            # ================= dense phase =================
            with tc.tile_pool(name="dsb", bufs=2) as dsb:
                for t in range(T):
                    n = rows(t)
                    xt = dsb.tile([128, KQ, 128], dt.float8e4, name="xt")
                    nc.sync.dma_start(
                        xt[:, :, :n],
                        xT.rearrange("(q p) m -> p q m", p=128)[:, :, 128 * t:128 * t + n])
                    xp_ps = proj(xt, n, wpT_s, IN, "xp", KQ)
                    xp = dsb.tile([128, IN], dt.bfloat16, name="xp")
                    ln_relu(xp_ps, n, IN, xp)
                    xpT = transpose_to(dsb, xp, n, IN, "xpT")

                    xl1_ps = proj(xpT, n, wl1_s, HID, "xl1", IN // 128)
                    xl1_bf = dsb.tile([128, HID], dt.bfloat16, name="xl1_bf")
                    nc.scalar.copy(xl1_bf[:n, :], xl1_ps[:n, :HID])
                    nc.sync.dma_start(xl1_own[128 * t:128 * t + n, :],
                                      xl1_bf[:n, :])

                    xr1_ps = proj(xpT, n, wr1_s, HID, "xr1", IN // 128)
                    nc.vector.tensor_copy(xr1_all[:n, t, :], xr1_ps[:n, :HID])

                    m1_ps = proj(xpT, n, wm1_s, HID, "m1", IN // 128)
                    m1 = dsb.tile([128, HID], dt.bfloat16, name="m1")
                    ln_relu(m1_ps, n, HID, m1)
                    m1T = transpose_to(dsb, m1, n, HID, "m1T")
                    zf_ps = proj(m1T, n, wm2_s, EMB, "zf", HID // 128)
                    zf_bf = dsb.tile([128, EMB], dt.bfloat16, name="zf_bf")
                    nc.vector.tensor_copy(zf_bf[:n, :], zf_ps[:n, :EMB])
                    nc.sync.dma_start(z_own[128 * t:128 * t + n, EMB:],
                                      zf_bf[:n, :])

            if stage >= 2:
                nc.gpsimd.collective_compute(
                    "AllGather", OP.bypass, replica_groups=rg,
                    ins=[xl1_own[:].opt()], outs=[xl1_tbl[:].opt()])

            # pad mask for ALL tiles in one op: mpad_all[p, t*KMAX+k] = (k >= deg[p,t])
            mpad_all = cpool.tile([128, T * KMAX], dt.float32, name="mpad_all")
            nc.vector.tensor_tensor(
                out=mpad_all[:],
                in0=bass.AP(iota_s.tensor, iota_s.offset,
                            [list(iota_s.ap[0]), [0, T], [1, KMAX]]),
                in1=bass.AP(deg_s.tensor, deg_s.offset,
                            [list(deg_s.ap[0]), [1, T], [0, KMAX]]),
                op=OP.is_ge)

            # ================= edge phase (CSR wide ops) =================
            def edge_tile(pools, t, xr_all, tbl, D, H, att_s, out_cb, suf):
                esb = pools["esb"]
                n = rows(t)
                Kt = int(K_t[t])
                c0 = int(OFF[t])
                DH = D // H
                psem = sems[suf]

                xg = esb.tile([128, Kt * D], dt.bfloat16, name=f"xg{suf}",
                              tag=f"xg{suf}", padded_shape=[128, KMAX * D])
                for k in range(Kt):
                    nc.gpsimd.indirect_dma_start(
                        out=xg[:, k * D:(k + 1) * D], out_offset=None,
                        in_=tbl[:],
                        in_offset=bass.IndirectOffsetOnAxis(
                            ap=idxe_s[:, c0 + k:c0 + k + 1], axis=0),
                    ).then_inc(psem, 16)
                gcnt[suf] += Kt
                nc.vector.tensor_copy(xg[:1, :1], xg[:1, :1])._wait_ge(
                    psem, 16 * gcnt[suf])

                # e = lrelu(xg + xr[dst]) ; score = <e, att> per head
                e_t = esb.tile([128, Kt * D], dt.bfloat16, name=f"e{suf}",
                               tag=f"e{suf}", padded_shape=[128, KMAX * D])
                xr_b = bass.AP(xr_all.tensor, xr_all.offset + t * D,
                               [list(xr_all.ap[0]), [0, Kt], [1, D]])
                nc.vector.tensor_tensor(out=e_t[:, :Kt * D],
                                        in0=xg[:, :Kt * D], in1=xr_b,
                                        op=OP.add)
                e2_t = esb.tile([128, Kt * D], dt.bfloat16, name=f"e2{suf}",
                                tag=f"e2{suf}", padded_shape=[128, KMAX * D])
                nc.vector.scalar_tensor_tensor(
                    out=e2_t[:, :Kt * D], in0=e_t[:, :Kt * D], scalar=0.2,
                    in1=e_t[:, :Kt * D], op0=OP.mult, op1=OP.max)
                att_b = bass.AP(att_s.tensor, att_s.offset,
                                [list(att_s.ap[0]), [0, Kt], [1, D]])
                sm_t = esb.tile([128, Kt * D], dt.bfloat16, name=f"smt{suf}",
                                tag=f"e{suf}", padded_shape=[128, KMAX * D])
                nc.vector.tensor_tensor(out=sm_t[:, :Kt * D],
                                        in0=e2_t[:, :Kt * D], in1=att_b,
                                        op=OP.mult)
                sc = esb.tile([128, Kt * H], dt.float32, name=f"sc{suf}",
                              tag=f"sc{suf}", padded_shape=[128, KMAX * H])
                nc.vector.tensor_reduce(
                    out=sc[:, :Kt * H],
                    in_=bass.AP(sm_t.tensor, sm_t.offset,
                                [list(sm_t.ap[0]), [DH, Kt * H], [1, DH]]),
                    axis=AX.X, op=OP.add)
                # mask pad slots: score += -100 * mpad   (broadcast over heads)
                mpad_b = bass.AP(mpad_all.tensor, mpad_all.offset + t * KMAX,
                                 [list(mpad_all.ap[0]), [1, Kt], [0, H]])
                nc.vector.scalar_tensor_tensor(
                    out=sc[:, :Kt * H], in0=mpad_b, scalar=-100.0,
                    in1=sc[:, :Kt * H], op0=OP.mult, op1=OP.add)
                ex = esb.tile([128, Kt * H], dt.float32, name=f"ex{suf}",
                              tag=f"ex{suf}", padded_shape=[128, KMAX * H])
                nc.scalar.activation(ex[:, :Kt * H], sc[:, :Kt * H], AF.Exp)
                den = sstat.tile([128, 8], dt.float32, name=f"den{suf}",
                                 tag=f"den{suf}")
                nc.vector.tensor_reduce(
                    out=den[:, :H],
                    in_=bass.AP(ex.tensor, ex.offset,
                                [list(ex.ap[0]), [1, H], [H, Kt]]),
                    axis=AX.X, op=OP.add)
                # no +eps: the self-loop term keeps den >= exp(score_self) > 0
                rec = sstat.tile([128, 8], dt.float32, name=f"rec{suf}",
                                 tag=f"rec{suf}")
                nc.vector.reciprocal(rec[:, :H], den[:, :H])
                alp = esb.tile([128, Kt * H], dt.bfloat16, name=f"al{suf}",
                               tag=f"al{suf}", padded_shape=[128, KMAX * H])
                rec_b = bass.AP(rec.tensor, rec.offset,
                                [list(rec.ap[0]), [0, Kt], [1, H]])
                nc.vector.tensor_tensor(out=alp[:, :Kt * H],
                                        in0=ex[:, :Kt * H], in1=rec_b,
                                        op=OP.mult)
                # w = xg * alpha ; out = sum_k w
                w_t = esb.tile([128, Kt * D], dt.bfloat16, name=f"w{suf}",
                               tag=f"e2{suf}", padded_shape=[128, KMAX * D])
                alp_b = bass.AP(alp.tensor, alp.offset,
                                [list(alp.ap[0]), [H, Kt], [1, H], [0, DH]])
                nc.vector.tensor_tensor(out=w_t[:, :Kt * D],
                                        in0=xg[:, :Kt * D], in1=alp_b,
                                        op=OP.mult)
                outf = esb.tile([128, D], dt.float32, name=f"o{suf}",
                                tag=f"o{suf}")
                nc.vector.tensor_reduce(
                    out=outf[:, :D],
                    in_=bass.AP(w_t.tensor, w_t.offset,
                                [list(w_t.ap[0]), [1, D], [D, Kt]]),
                    axis=AX.X, op=OP.add)
                out_cb(pools, outf, n, t)

            def l1_out(pools, outf, n, t):
                esb = pools["esb"]
                h_bf = esb.tile([128, HID], dt.bfloat16, name="h_bf",
                                tag="h_bf")
                ln_relu(outf, n, HID, h_bf)
                hT = transpose_to(esb, h_bf, n, HID, "hT")
                xl2_ps = proj(hT, n, wl2_s, EMB, "xl2", HID // 128)
                xl2_bf = esb.tile([128, EMB], dt.bfloat16, name="xl2_bf",
                                  tag="xl2_bf")
                nc.scalar.copy(xl2_bf[:n, :], xl2_ps[:n, :EMB])
                nc.sync.dma_start(xl2_own[128 * t:128 * t + n, :],
                                  xl2_bf[:n, :])
                xr2_ps = proj(hT, n, wr2_s, EMB, "xr2", HID // 128)
                nc.vector.tensor_copy(xr2_all[:n, t, :], xr2_ps[:n, :EMB])

            def l2_out(pools, outf, n, t):
                esb = pools["esb"]
                zg = esb.tile([128, EMB], dt.bfloat16, name="zg", tag="zg")
                nc.vector.tensor_copy(zg[:n, :], outf[:n, :EMB])
                nc.sync.dma_start(z_own[128 * t:128 * t + n, :EMB], zg[:n, :])

            if stage >= 3:
                with tc.tile_pool(name="esb_a", bufs=1) as esb_a:
                    pools = {"esb": esb_a}
                    for t in range(T):
                        edge_tile(pools, t, xr1_all, xl1_tbl, HID, 4, att1_s,
                                  l1_out, "a")

            if stage >= 4:
                nc.gpsimd.collective_compute(
                    "AllGather", OP.bypass, replica_groups=rg,
                    ins=[xl2_own[:].opt()], outs=[xl2_tbl[:].opt()])

                with tc.tile_pool(name="esb_b", bufs=1) as esb_b:
                    pools = {"esb": esb_b}
                    for t in range(T):
                        edge_tile(pools, t, xr2_all, xl2_tbl, EMB, 1, att2_s,
                                  l2_out, "b")

                nc.gpsimd.collective_compute(
                    "AllGather", OP.bypass, replica_groups=rg,
                    ins=[z_own[:].opt()], outs=[z_tbl[:].opt()])

            # ================= decode =================
            D2 = 2 * EMB
            NCOL = cfg.PPC // 128          # 256
            CC = 32                        # columns per chunk
            res_sb = cpool.tile([128, NCOL], dt.float32, name="res_sb")
            if stage < 5:
                nc.vector.memset(res_sb[:], 0.0)
            with tc.tile_pool(name="dec", bufs=1) as dec:
                if stage >= 5:
                    pi_t = cpool.tile([128, NCOL], dt.int32, name="pi")
                    nc.sync.dma_start(pi_t[:], PS32)
                    pj_t = cpool.tile([128, NCOL], dt.int32, name="pj")
                    nc.sync.dma_start(pj_t[:], PD32)
                for ch in range(NCOL // CC if stage >= 5 else 0):
                    o = ch * CC
                    za = dec.tile([128, CC * D2], dt.bfloat16, name="za",
                                  tag="za")
                    zb = dec.tile([128, CC * D2], dt.bfloat16, name="zb",
                                  tag="zb")
                    for j in range(CC):
                        nc.gpsimd.indirect_dma_start(
                            out=za[:, j * D2:(j + 1) * D2], out_offset=None,
                            in_=z_tbl[:],
                            in_offset=bass.IndirectOffsetOnAxis(
                                ap=pi_t[:, o + j:o + j + 1], axis=0),
                        ).then_inc(sems["d"], 16)
                        nc.gpsimd.indirect_dma_start(
                            out=zb[:, j * D2:(j + 1) * D2], out_offset=None,
                            in_=z_tbl[:],
                            in_offset=bass.IndirectOffsetOnAxis(
                                ap=pj_t[:, o + j:o + j + 1], axis=0),
                        ).then_inc(sems["d"], 16)
                    gcnt["d"] += 2 * CC
                    nc.vector.tensor_copy(za[:1, :1], za[:1, :1])._wait_ge(
                        sems["d"], 16 * gcnt["d"])
                    nc.vector.tensor_copy(zb[:1, :1], zb[:1, :1])._wait_ge(
                        sems["d"], 16 * gcnt["d"])

                    prod = dec.tile([128, CC * D2], dt.float32, name="prod",
                                    tag="prod")
                    view = lambda t_: bass.AP(
                        t_.tensor, t_.offset,
                        [list(t_.ap[0]), [EMB, CC * 2], [1, EMB]])
                    dots = dec.tile([128, CC * 2], dt.float32, name="dots",
                                    tag="dots")
                    nc.vector.tensor_tensor(out=prod[:], in0=za[:], in1=zb[:],
                                            op=OP.mult)
                    nc.vector.tensor_reduce(out=dots[:], in_=view(prod),
                                            axis=AX.X, op=OP.add)
                    sqa = dec.tile([128, CC * 2], dt.float32, name="sqa",
                                   tag="sqa")
                    nc.vector.tensor_tensor(out=prod[:], in0=za[:], in1=za[:],
                                            op=OP.mult)
                    nc.vector.tensor_reduce(out=sqa[:], in_=view(prod),
                                            axis=AX.X, op=OP.add)
                    sqb = dec.tile([128, CC * 2], dt.float32, name="sqb",
                                   tag="sqb")
                    nc.vector.tensor_tensor(out=prod[:], in0=zb[:], in1=zb[:],
                                            op=OP.mult)
                    nc.vector.tensor_reduce(out=sqb[:], in_=view(prod),
                                            axis=AX.X, op=OP.add)
                    nn_ = dec.tile([128, CC * 2], dt.float32, name="nn_",
                                   tag="nn_")
                    nc.vector.tensor_tensor(out=nn_[:], in0=sqa[:],
                                            in1=sqb[:], op=OP.mult)
                    rin = dec.tile([128, CC * 2], dt.float32, name="rin",
                                   tag="rin")
                    nc.vector.reciprocal(rin[:], nn_[:])
                    rsq = dec.tile([128, CC * 2], dt.float32, name="rsq",
                                   tag="rsq")
                    nc.scalar.activation(rsq[:], rin[:], AF.Sqrt)
                    cosv = dec.tile([128, CC * 2], dt.float32, name="cosv",
                                    tag="cosv")
                    nc.vector.tensor_tensor(out=cosv[:], in0=dots[:],
                                            in1=rsq[:], op=OP.mult)
                    wz = dec.tile([128, CC * 2], dt.float32, name="wz",
                                  tag="wz")
                    a12b = bass.AP(a12_s.tensor, a12_s.offset,
                                   [list(a12_s.ap[0]), [0, CC], [1, 2]])
                    nc.vector.tensor_tensor(out=wz[:], in0=cosv[:], in1=a12b,
                                            op=OP.mult)
                    nc.vector.tensor_reduce(
                        out=res_sb[:, o:o + CC],
                        in_=bass.AP(wz.tensor, wz.offset,
                                    [list(wz.ap[0]), [2, CC], [1, 2]]),
                        axis=AX.X, op=OP.add)

            nc.sync.dma_start(res_out.rearrange("(a b) -> b a", b=128),
                              res_sb[:])

    nc.compile()
    return nc


# ---------------------------------------------------------------------------
# entry point
# ---------------------------------------------------------------------------

def make_in_maps(plan, W, cfg):
    in_maps = []
    CKU = None
    for c in range(cfg.NC):
        m = {"xT": plan.xT[c], "DEGT": plan.DEGT[c],
             "PS32": plan.PS[c], "PD32": plan.PD[c],
             "WBLOB": W["WBLOB"][c]}
        for k in ("ATT1R", "ATT2R", "IDENT", "IOTA_ROWS", "A12R"):
            m[k] = W[k]
        in_maps.append(m)
    return in_maps


def finish_in_maps(in_maps, plan, cfg, nc):
    """Re-pack IDXE per core to the unified per-tile offsets of the program."""
    K_t = np.stack([plan.K_t[c] for c in range(cfg.NC)]).max(axis=0)
    OFF = np.concatenate([[0], np.cumsum(K_t)]).astype(np.int64)
    CKU = int(OFF[-1])
    for c in range(cfg.NC):
        idxe = np.zeros((128, CKU), dtype=np.int32)
        for t in range(cfg.T):
            kc = int(plan.K_t[c][t])
            oc = int(plan.OFF_t[c][t])
            idxe[:, int(OFF[t]):int(OFF[t]) + kc] = \
                plan.IDXE[c][:, oc:oc + kc]
        in_maps[c]["IDXE"] = idxe
    return in_maps


def kernel(**inputs):
    cfg = CFG
    plan = host_prep(inputs["x"], inputs["edge_index"],
                     inputs["edge_pairs"], cfg)
    W = prep_weights(inputs, cfg)
    nc = build_program(plan, cfg)
    from concourse.bass_utils import run_bass_kernel_spmd
    in_maps = finish_in_maps(make_in_maps(plan, W, cfg), plan, cfg, nc)
    res = run_bass_kernel_spmd(nc, in_maps, core_ids=list(range(cfg.NC)))
    out = np.concatenate([np.asarray(res.results[c]["res"])
                          for c in range(cfg.NC)])
    return out.astype(np.float32)
